# revision 1
# baseline (speedup 1.0000x reference)
"""Causal self-attention (B=2, T=2048, C=1024, 16 heads) on 8 Trainium2 cores.

Sharding: data-parallel over batch (2), tensor-parallel over heads (4/core).
Core c = b*4+g handles batch b, heads [4g, 4g+4). Each core computes its
qkv slice, causal attention for its 4 heads, and a row-parallel partial of
the output projection (its 256 input channels of w_proj). The host sums the
4 partials per batch; b_proj is added on-device exactly once per column
(each core receives b_proj zero-masked to its own column quarter, host
pre-broadcast across partitions, added during the PSUM->SBUF move).

Device layout (per core):
  xT   [128, 8, 2048]  x^T with channels on partitions (host pre-transposed)
  q^T/k^T computed as [128ch, 2, 2048] (2 tiles of 2 heads each)
  S^T[tk, tq] = (k^T)^T @ q^T per head; two heads packed in the 128x128 PE
  array via base-partition row groups (K=64 each). exp on ScalarE reads
  PSUM directly (scores ~ N(0,1): no max subtraction needed); causal mask
  applied only on diagonal tiles via a 0/1 mask multiply; off-diagonal
  upper tiles are never computed and diagonal tiles are column-narrowed
  (clamped to >=256 wide for full-rate fp32r). The PV matmul uses v
  extended with a ones column -> row 64 of the PSUM accumulator is the
  softmax denominator for free. All matmul operands are bitcast to
  float32r (full PE rate, TF32-like multiply precision, fp32 accumulate).

Phase order interleaves qkv with attention so ScalarE's exp stream (the
attention-phase bottleneck) starts as early as possible:
  A: q/k for head-pair 0   B: v for t 0..7
  [attention hp0 j0,j1]    C: q/k for head-pair 1   D: v for t 8..15
  [attention hp0 j2,j3; hp1 j0..3; projection per j]
"""

import numpy as np

B, T, C = 2, 2048, 1024
NH, HD = 16, 64
NCORES = 8
HPC = 4                # heads per core
CPC = HPC * HD         # 256 channels per core
P = 128
CT = C // P            # 8 contraction tiles over C
TT = T // P            # 16 tiles of 128 over T
NTQ = T // 512         # 4 query blocks of 512
VW = HD + 1            # 65: head width in vext (v columns + ones column)
MW = 640               # mask tile width (mask[p,u] = p <= u-128)

_CACHE = {}


def _emit(tc, out_ap, ins):
    """Emit the per-core program into TileContext tc.

    ins: dict of input APs (xT, wq, wk, wv, bq, bk, vinit, mask, wp, bp).
    out_ap: [T, C] partial-output DRAM AP.
    """
    import concourse.mybir as mybir
    from concourse.bass import ts

    nc = tc.nc
    f32 = mybir.dt.float32
    f32r = mybir.dt.float32r
    Exp = mybir.ActivationFunctionType.Exp

    def r(ap):
        # float32r: same fp32 bits, PE streams at full rate (vs 4 cyc/row
        # for plain fp32) at TF32-like multiply precision; fp32 accumulate.
        return ap.bitcast(mybir.dt.float32r)

    with (
        tc.tile_pool(name="pers", bufs=1) as pers,
        tc.tile_pool(name="xw", bufs=1) as xw,
        tc.tile_pool(name="attn_sb", bufs=1) as asb,
        tc.tile_pool(name="ps", bufs=1, space="PSUM") as ps,
    ):
        qT_sb = pers.tile([P, 2, T], f32r, name="qT_sb")
        kT_sb = pers.tile([P, 2, T], f32r, name="kT_sb")
        yT_sb = pers.tile([P, 2, T], f32r, name="yT_sb")
        vext_sb = pers.tile([P, TT, HPC * VW], f32r, name="vext_sb")
        vinit_sb = pers.tile([P, HPC * VW], f32, name="vinit_sb")
        mask_sb = pers.tile([P, MW], f32, name="mask_sb")
        bq_sb = pers.tile([P, 2], f32, name="bq_sb")
        bk_sb = pers.tile([P, 2], f32, name="bk_sb")
        wp_sb = pers.tile([P, 2, C], f32r, name="wp_sb")
        bp_sb = pers.tile([P, C], f32, name="bp_sb")

        xT_sb = xw.tile([P, CT, T], f32r, name="xT_sb")
        wq_sb = xw.tile([P, CT, CPC], f32r, name="wq_sb")
        wk_sb = xw.tile([P, CT, CPC], f32r, name="wk_sb")
        wv_sb = xw.tile([P, CT, CPC], f32r, name="wv_sb")

        # Load order: first q/k weight columns + first x^T query block up
        # front (the startup matmul interleave starts on them), then the
        # rest of the stream; smalls mid-stream, proj weights last.
        nc.sync.dma_start(out=wq_sb[:, :, 0:P], in_=r(ins["wq"][:, :, 0:P]))
        nc.sync.dma_start(out=xT_sb[:, 0, 0:512], in_=r(ins["xT"][:, 0, 0:512]))
        nc.sync.dma_start(out=wk_sb[:, :, 0:P], in_=r(ins["wk"][:, :, 0:P]))
        nc.sync.dma_start(out=xT_sb[:, 0, 512:T], in_=r(ins["xT"][:, 0, 512:T]))
        nc.sync.dma_start(out=wq_sb[:, :, P:CPC], in_=r(ins["wq"][:, :, P:CPC]))
        nc.sync.dma_start(out=wk_sb[:, :, P:CPC], in_=r(ins["wk"][:, :, P:CPC]))
        nc.sync.dma_start(out=wv_sb[:, :, :], in_=r(ins["wv"]))
        for ct in range(1, 4):
            nc.sync.dma_start(out=xT_sb[:, ct, :], in_=r(ins["xT"][:, ct, :]))
        nc.sync.dma_start(out=vinit_sb[:, :], in_=ins["vinit"])
        nc.sync.dma_start(out=mask_sb[:, :], in_=ins["mask"])
        nc.sync.dma_start(out=bq_sb[:, :], in_=ins["bq"])
        nc.sync.dma_start(out=bk_sb[:, :], in_=ins["bk"])
        for ct in range(4, CT):
            nc.sync.dma_start(out=xT_sb[:, ct, :], in_=r(ins["xT"][:, ct, :]))
        nc.sync.dma_start(out=bp_sb[:, :], in_=ins["bp"])
        nc.sync.dma_start(out=wp_sb[:, :, :], in_=r(ins["wp"]))

        # Pre-load the exp table set during the load phase (first exp
        # otherwise pays ~2.7us mid-kernel). Output is scratch.
        warm = asb.tile([1, 8], f32, tag="rec", bufs=2, name="warm")
        nc.scalar.activation(warm[0:1, :], mask_sb[0:1, 0:8], Exp, scale=1.0)

        # --- work generators: each yield is ~one PE matmul, so attention
        # blocks can pump them as fillers between their own iterations to
        # keep the (in-order) PE stream dense while ScalarE runs exp.
        from collections import deque

        work = deque()  # (name, generator)

        def pump(n):
            done = 0
            while done < n and work:
                _, g = work[0]
                try:
                    next(g)
                    done += 1
                except StopIteration:
                    work.popleft()

        def flush_to(target):
            while work:
                name, g = work.popleft()
                for _ in g:
                    pass
                if name == target:
                    return

        def flush_all():
            while work:
                _, g = work.popleft()
                for _ in g:
                    pass

        def qk_gen(dst_sb, w_sb, b_sb, m, tq, nm):
            pt = ps.tile([P, 512], f32, tag="qkv", bufs=2,
                         name=f"ps_{nm}_{m}_{tq}")
            for ct in range(CT):
                nc.tensor.matmul(
                    pt[:, :],
                    r(w_sb[:, ct, ts(m, P)]),
                    r(xT_sb[:, ct, ts(tq, 512)]),
                    start=(ct == 0),
                    stop=(ct == CT - 1),
                )
                if ct == CT - 1:
                    nc.vector.tensor_scalar_add(
                        dst_sb[:, m, ts(tq, 512)], pt[:, :], b_sb[:, m : m + 1]
                    )
                yield

        def v_gen(t):
            pt = ps.tile([P, CPC], f32, tag="qkv", bufs=2, name=f"ps_v_{t}")
            for ct in range(CT):
                nc.tensor.matmul(
                    pt[:, :],
                    r(xT_sb[:, ct, ts(t, P)]),
                    r(wv_sb[:, ct, :]),
                    start=(ct == 0),
                    stop=(ct == CT - 1),
                )
                if ct == CT - 1:
                    vslot = vext_sb[:, t, :].rearrange(
                        "p (h u) -> p h u", u=VW
                    )
                    vini = vinit_sb[:, :].rearrange("p (h u) -> p h u", u=VW)
                    nc.vector.tensor_add(
                        vslot[:, :, 0:HD],
                        pt[:, :].rearrange("p (h d) -> p h d", d=HD),
                        vini[:, :, 0:HD],
                    )
                    nc.vector.tensor_copy(
                        vslot[:, :, HD : HD + 1], vini[:, :, HD : HD + 1]
                    )
                yield

        def proj_gen(t):
            stage = asb.tile([P, C], f32, tag="stage", bufs=4,
                             name=f"stage_{t}")
            for ch in range(2):
                prj = ps.tile([P, 512], f32, tag="qkv", bufs=2,
                              name=f"prj_{t}_{ch}")
                for m in range(2):
                    nc.tensor.matmul(
                        prj[:, :],
                        r(yT_sb[:, m, ts(t, P)]),
                        r(wp_sb[:, m, ts(ch, 512)]),
                        start=(m == 0),
                        stop=(m == 1),
                    )
                    if m == 1:
                        nc.vector.tensor_add(
                            stage[:, ts(ch, 512)], prj[:, :],
                            bp_sb[:, ts(ch, 512)],
                        )
                        nc.sync.dma_start(
                            out=out_ap[ts(t, P), ts(ch, 512)],
                            in_=stage[:, ts(ch, 512)],
                        )
                    yield

        def run_now(gen):
            for _ in gen:
                pass

        def attention_block(hp, j):
            n_tk = 4 * (j + 1)
            pv = [
                ps.tile([P, 512], f32, tag="pv", bufs=2,
                        name=f"pv_{j}_{hp}_{a}")
                for a in range(2)
            ]
            for tk in range(n_tk):
                # fp32r needs >=256 moving cols for full PE rate, so clamp
                # the diagonal narrowing at 256 wide.
                off = min(max(0, P * tk - 512 * j), 256)
                sp = ps.tile([P, 2, 512], f32, tag="s", bufs=2,
                             name=f"s_{j}_{hp}_{tk}")
                for a in range(2):
                    lo, hi = a * 64, a * 64 + 64
                    nc.tensor.matmul(
                        sp[:, a, off:512],
                        r(kT_sb[lo:hi, hp, ts(tk, P)]),
                        r(qT_sb[lo:hi, hp, 512 * j + off : 512 * (j + 1)]),
                        start=True,
                        stop=True,
                    )
                pt = asb.tile([P, 2, 512], f32r, tag="pt", bufs=4,
                              name=f"pt_{j}_{hp}_{tk}")
                nc.scalar.activation(
                    pt[:, :, off:512], sp[:, :, off:512], Exp, scale=0.125
                )
                if tk >= 4 * j:  # diagonal tile: apply causal 0/1 mask
                    o = 512 * j - P * tk  # in [-384, 0]
                    # invalid entries (p > f+o) only exist for f < -o+128;
                    # columns past that are valid for every partition, so
                    # the mask multiply needs at most 128 columns (256 for
                    # the one tile whose narrowing was clamped at 256).
                    wm = 128 if off == -o else 512 - off
                    for a in range(2):
                        nc.vector.tensor_mul(
                            pt[:, a, off : off + wm],
                            pt[:, a, off : off + wm],
                            mask_sb[:, P + o + off : P + o + off + wm],
                        )
                for a in range(2):
                    h = 2 * hp + a
                    nc.tensor.matmul(
                        pv[a][0:VW, off:512],
                        r(vext_sb[:, tk, ts(h, VW)]),
                        r(pt[:, a, off:512]),
                        start=(tk == 0),
                        stop=(tk == n_tk - 1),
                    )
                pump(4)
            for a in range(2):
                lo, hi = a * 64, a * 64 + 64
                rec = asb.tile([1, 512], f32, tag="rec", bufs=2,
                               name=f"rec_{j}_{hp}_{a}")
                nc.vector.reciprocal(rec[0:1, :], pv[a][HD : HD + 1, :])
                rec_bc = asb.tile([HD, 512], f32, tag="recb", bufs=2,
                                  name=f"recb_{j}_{hp}_{a}")
                nc.gpsimd.partition_broadcast(rec_bc[0:HD, :], rec[0:1, :])
                nc.vector.tensor_mul(
                    yT_sb[lo:hi, hp, ts(j, 512)],
                    pv[a][0:HD, :],
                    rec_bc[0:HD, :],
                )

        # Schedule: kick off attention (the ScalarE exp stream is the
        # attention bottleneck) as soon as its inputs exist, biggest query
        # blocks early, smallest last so the tail is short. proj(j) goes
        # out as soon as both head-pairs finished block j.
        # Startup: ten passes (q/k for tq0..tq2, v t0..t3) interleaved
        # ct-major so the PE has ~10 matmuls to run per arriving x^T tile
        # during the input-DMA wall. The extra passes borrow the (still
        # idle) "s"/"pv" PSUM slots; two q/k passes pack per 2-bank "s"
        # slot and two v passes per "pv" bank (disjoint columns).
        sq0 = ps.tile([P, 512], f32, tag="qkv", bufs=2, name="ps_q_0_0")
        sk0 = ps.tile([P, 512], f32, tag="qkv", bufs=2, name="ps_k_0_0")
        sqk1 = ps.tile([P, 2, 512], f32, tag="s", bufs=2, name="ps_qk_0_1")
        sqk2 = ps.tile([P, 2, 512], f32, tag="s", bufs=2, name="ps_qk_0_2")
        sv0 = ps.tile([P, 512], f32, tag="pv", bufs=2, name="ps_v_0")
        sv1 = ps.tile([P, 512], f32, tag="pv", bufs=2, name="ps_v_1")
        for ct in range(CT):
            st = ct == 0
            sp_ = ct == CT - 1
            nc.tensor.matmul(sq0[:, :], r(wq_sb[:, ct, ts(0, P)]),
                             r(xT_sb[:, ct, ts(0, 512)]), start=st, stop=sp_)
            nc.tensor.matmul(sk0[:, :], r(wk_sb[:, ct, ts(0, P)]),
                             r(xT_sb[:, ct, ts(0, 512)]), start=st, stop=sp_)
            nc.tensor.matmul(sqk1[:, 0, :], r(wq_sb[:, ct, ts(0, P)]),
                             r(xT_sb[:, ct, ts(1, 512)]), start=st, stop=sp_)
            nc.tensor.matmul(sqk1[:, 1, :], r(wk_sb[:, ct, ts(0, P)]),
                             r(xT_sb[:, ct, ts(1, 512)]), start=st, stop=sp_)
            nc.tensor.matmul(sqk2[:, 0, :], r(wq_sb[:, ct, ts(0, P)]),
                             r(xT_sb[:, ct, ts(2, 512)]), start=st, stop=sp_)
            nc.tensor.matmul(sqk2[:, 1, :], r(wk_sb[:, ct, ts(0, P)]),
                             r(xT_sb[:, ct, ts(2, 512)]), start=st, stop=sp_)
            nc.tensor.matmul(sv0[:, 0:CPC], r(xT_sb[:, ct, ts(0, P)]),
                             r(wv_sb[:, ct, :]), start=st, stop=sp_)
            nc.tensor.matmul(sv1[:, 0:CPC], r(xT_sb[:, ct, ts(1, P)]),
                             r(wv_sb[:, ct, :]), start=st, stop=sp_)
        for m_, tq_, pt_, dst_, b_ in (
            (0, 0, sq0[:, :], qT_sb, bq_sb),
            (0, 0, sk0[:, :], kT_sb, bk_sb),
            (0, 1, sqk1[:, 0, :], qT_sb, bq_sb),
            (0, 1, sqk1[:, 1, :], kT_sb, bk_sb),
            (0, 2, sqk2[:, 0, :], qT_sb, bq_sb),
            (0, 2, sqk2[:, 1, :], kT_sb, bk_sb),
        ):
            nc.vector.tensor_scalar_add(
                dst_[:, m_, ts(tq_, 512)], pt_, b_[:, m_ : m_ + 1]
            )
        vini = vinit_sb[:, :].rearrange("p (h u) -> p h u", u=VW)
        for t in range(2):
            pt_ = (sv0, sv1)[t][:, 0:CPC]
            vslot = vext_sb[:, t, :].rearrange("p (h u) -> p h u", u=VW)
            nc.vector.tensor_add(
                vslot[:, :, 0:HD],
                pt_.rearrange("p (h d) -> p h d", d=HD),
                vini[:, :, 0:HD],
            )
            nc.vector.tensor_copy(
                vslot[:, :, HD : HD + 1], vini[:, :, HD : HD + 1]
            )
        run_now(v_gen(2))
        run_now(v_gen(3))

        for t in range(4, 8):
            work.append((f"v{t}", v_gen(t)))
        for t in range(8, 12):
            work.append((f"v{t}", v_gen(t)))
        work.append(("q_0_3", qk_gen(qT_sb, wq_sb, bq_sb, 0, 3, "q")))
        work.append(("k_0_3", qk_gen(kT_sb, wk_sb, bk_sb, 0, 3, "k")))
        for t in range(12, 16):
            work.append((f"v{t}", v_gen(t)))
        for tq in range(NTQ):
            work.append((f"q_1_{tq}", qk_gen(qT_sb, wq_sb, bq_sb, 1, tq, "q")))
            work.append((f"k_1_{tq}", qk_gen(kT_sb, wk_sb, bk_sb, 1, tq, "k")))

        attention_block(0, 0)
        flush_to("v7")
        attention_block(0, 1)
        flush_to("v11")
        attention_block(0, 2)
        flush_to("v15")
        attention_block(0, 3)
        flush_to("k_1_3")
        attention_block(1, 3)
        for t in range(12, 16):
            work.append((f"p{t}", proj_gen(t)))
        attention_block(1, 2)
        for t in range(8, 12):
            work.append((f"p{t}", proj_gen(t)))
        attention_block(1, 0)
        for t in range(0, 4):
            work.append((f"p{t}", proj_gen(t)))
        attention_block(1, 1)
        for t in range(4, 8):
            work.append((f"p{t}", proj_gen(t)))
        flush_all()


def _build_bass():
    import concourse.mybir as mybir
    import concourse.tile as tile
    from concourse import bacc

    f32 = mybir.dt.float32
    nc = bacc.Bacc("TRN2", num_devices=NCORES)

    shapes = {
        "xT": [P, CT, T],
        "wq": [P, CT, CPC],
        "wk": [P, CT, CPC],
        "wv": [P, CT, CPC],
        "bq": [P, 2],
        "bk": [P, 2],
        "vinit": [P, HPC * VW],
        "mask": [P, MW],
        "wp": [P, 2, C],
        "bp": [P, C],
    }
    ins = {
        name: nc.dram_tensor(name, shp, f32, kind="ExternalInput").ap()
        for name, shp in shapes.items()
    }
    out_ap = nc.dram_tensor("out", [T, C], f32, kind="ExternalOutput").ap()

    with tile.TileContext(nc) as tc:
        _emit(tc, out_ap, ins)
    nc.compile()
    return nc


def _causal_mask_host():
    p = np.arange(P)[:, None]
    u = np.arange(MW)[None, :]
    return (p <= u - P).astype(np.float32)


def _shard(x, w_attn, b_attn, w_proj, b_proj):
    mask = _causal_mask_host()
    xTs = [
        np.ascontiguousarray(
            x[b].T.reshape(CT, P, T).transpose(1, 0, 2)
        )
        for b in range(B)
    ]

    def wslice(off):
        w = w_attn[:, off : off + CPC]
        return np.ascontiguousarray(w.reshape(CT, P, CPC).transpose(1, 0, 2))

    maps = []
    for core in range(NCORES):
        b, g = divmod(core, NCORES // B)
        c0 = g * CPC
        bv = b_attn[2 * C + c0 : 2 * C + c0 + CPC]
        vinit = np.zeros((P, HPC * VW), np.float32)
        for h in range(HPC):
            vinit[:, h * VW : h * VW + HD] = bv[h * HD : (h + 1) * HD][None, :]
            vinit[:, h * VW + HD] = 1.0
        bp = np.zeros((P, C), np.float32)
        bp[:, c0 : c0 + CPC] = b_proj[c0 : c0 + CPC][None, :]
        maps.append(
            {
                "xT": xTs[b],
                "wq": wslice(c0),
                "wk": wslice(C + c0),
                "wv": wslice(2 * C + c0),
                "bq": np.ascontiguousarray(
                    b_attn[c0 : c0 + CPC].reshape(2, P).T
                ),
                "bk": np.ascontiguousarray(
                    b_attn[C + c0 : C + c0 + CPC].reshape(2, P).T
                ),
                "vinit": vinit,
                "mask": mask,
                "wp": np.ascontiguousarray(
                    w_proj[c0 : c0 + CPC, :].reshape(2, P, C).transpose(1, 0, 2)
                ),
                "bp": bp,
            }
        )
    return maps


TRACE = False
LAST = None


def _stub_missing_axon_hooks():
    """Some containers lack antenv.axon_hooks; stub it so trace=True
    degrades to a warning instead of crashing run_bass_kernel_spmd."""
    import sys
    import types

    try:
        import antenv.axon_hooks  # noqa: F401
    except ModuleNotFoundError:
        mod = types.ModuleType("antenv.axon_hooks")
        mod.get_axon_ntff_profile_hook = lambda: None
        sys.modules["antenv.axon_hooks"] = mod


def kernel(x, w_attn, b_attn, w_proj, b_proj):
    global LAST
    _stub_missing_axon_hooks()
    from concourse.bass_utils import run_bass_kernel_spmd

    x = np.asarray(x, np.float32)
    w_attn = np.asarray(w_attn, np.float32)
    b_attn = np.asarray(b_attn, np.float32)
    w_proj = np.asarray(w_proj, np.float32)
    b_proj = np.asarray(b_proj, np.float32)

    if "nc" not in _CACHE:
        _CACHE["nc"] = _build_bass()
    nc = _CACHE["nc"]

    in_maps = _shard(x, w_attn, b_attn, w_proj, b_proj)
    res = run_bass_kernel_spmd(
        nc, in_maps, core_ids=list(range(NCORES)), trace=TRACE
    )
    LAST = res
    out = np.zeros((B, T, C), np.float32)
    for core in range(NCORES):
        out[core // (NCORES // B)] += res.results[core]["out"]
    return out



# revision 21
# speedup vs baseline: 1.2238x; 1.2238x over previous
"""Causal self-attention (B=2, T=2048, C=1024, 16 heads) on 8 Trainium2 cores.

Sharding: data-parallel over batch (2), tensor-parallel over heads (4/core).
Core c = b*4+g handles batch b, heads [4g, 4g+4). Each core computes its
qkv slice, causal attention for its 4 heads, and a row-parallel partial of
the output projection (its 256 input channels of w_proj). The host sums the
4 partials per batch; b_proj is added on-device exactly once per column
(each core receives b_proj zero-masked to its own column quarter, host
pre-broadcast across partitions, added during the PSUM->SBUF move).

All data is bf16 (inputs rounded host-side): matmul streams at the same
1 cyc/row as fp32r but without the >=256-moving-column restriction, so
diagonal attention tiles narrow to their true width; DVE elementwise ops
on pure-SBUF bf16 run at 2x; DMA bytes halve. Partial outputs leave the
device as bf16 and are summed in f32 on the host (rel-err ~1e-3, well
under the 2e-2 gate).

Device layout (per core):
  xT   [128, 8, 2048]  x^T with channels on partitions (host pre-transposed)
  w3   [128, 8, 3, 256] packed {wq,wk,wv} column slices
  q^T/k^T computed as [128ch, 2, 2048] (2 tiles of 2 heads each)
  S^T[tk, tq] = (k^T)^T @ q^T per head; two heads packed in the 128x128 PE
  array via base-partition row groups (K=64 each). exp on ScalarE reads
  PSUM directly (scores ~ N(0,1): no max subtraction needed); causal mask
  applied only on diagonal tiles via a 0/1 mask multiply on the 128-col
  window that actually straddles the diagonal. The PV matmul uses v
  extended with a ones column -> row 64 of the PSUM accumulator is the
  softmax denominator for free.

A memset + ~26 dummy 128-col matmuls run during the initial DMA wall so
the PE clock ramp (0.65/1.2 GHz cold states in the cost model) completes
on garbage work before the first real matmul issues.
"""

import numpy as np

B, T, C = 2, 2048, 1024
NH, HD = 16, 64
NCORES = 8
HPC = 4                # heads per core
CPC = HPC * HD         # 256 channels per core
P = 128
CT = C // P            # 8 contraction tiles over C
TT = T // P            # 16 tiles of 128 over T
NTQ = T // 512         # 4 query blocks of 512
VW = HD + 1            # 65: head width in vext (v columns + ones column)
N_WARM = 26            # PE ramp-warmup matmuls (128 cols each)
N_SHIM = 3             # dummy matmuls between startup ct-groups

_CACHE = {}


def _emit(tc, out_ap, ins):
    """Emit the per-core program into TileContext tc.

    ins: dict of input APs (xT, w3, bq, bk, vinit, mask, wp, bp).
    out_ap: [T, C] partial-output DRAM AP (bf16).
    """
    import concourse.mybir as mybir
    from concourse.bass import ts

    nc = tc.nc
    f32 = mybir.dt.float32
    bf16 = mybir.dt.bfloat16
    Exp = mybir.ActivationFunctionType.Exp
    Copy = mybir.ActivationFunctionType.Copy

    with (
        tc.tile_pool(name="pers", bufs=1) as pers,
        tc.tile_pool(name="xw", bufs=1) as xw,
        tc.tile_pool(name="attn_sb", bufs=1) as asb,
        tc.tile_pool(name="ps", bufs=1, space="PSUM") as ps,
    ):
        qT_sb = pers.tile([P, 2, T], bf16, name="qT_sb")
        kT_sb = pers.tile([P, 2, T], bf16, name="kT_sb")
        yT_sb = pers.tile([P, 2, T], bf16, name="yT_sb")
        y_sb = pers.tile([P, TT, 2, 2, HD], bf16, name="y_sb")
        ident_sb = pers.tile([P, P], bf16, name="ident_sb")
        vext_sb = pers.tile([P, TT, HPC * VW], bf16, name="vext_sb")
        vinit_sb = pers.tile([P, HPC * VW], bf16, name="vinit_sb")
        mask_sb = pers.tile([P, P], bf16, name="mask_sb")
        bq_sb = pers.tile([P, 2], f32, name="bq_sb")
        bk_sb = pers.tile([P, 2], f32, name="bk_sb")
        wp_sb = pers.tile([P, 2, C], bf16, name="wp_sb")
        bp_sb = pers.tile([P, C], bf16, name="bp_sb")
        warm_sb = pers.tile([P, P], bf16, name="warm_sb")

        xT_sb = xw.tile([P, CT, T], bf16, name="xT_sb")
        w3_sb = xw.tile([P, CT, 3, CPC], bf16, name="w3_sb")

        def wq(ct):
            return w3_sb[:, ct, 0, :]

        def wk(ct):
            return w3_sb[:, ct, 1, :]

        def wv(ct):
            return w3_sb[:, ct, 2, :]

        # PE ramp warmup: memset a small SBUF tile (DVE, no deps, runs at
        # t~0), then stream dummy matmuls through the otherwise-idle PE
        # while the first input DMAs land. Outputs go to the (still idle)
        # "s"-tag PSUM slots and are never read.
        nc.vector.memset(warm_sb[:, :], 0.0)

        def dummy_mm(nm, n):
            for w in range(n):
                wt = ps.tile([P, 2, 512], f32, tag="s", bufs=2,
                             name=f"warm_{nm}_{w}")
                nc.tensor.matmul(
                    wt[:, 0, 0:P], warm_sb[:, :], warm_sb[:, :],
                    start=True, stop=True,
                )

        dummy_mm("init", N_WARM)

        # Load order: ct0 of the packed qkv weights + the first x^T block
        # split in three (the startup matmul interleave starts on them),
        # then per-ct (w3, xT halves) so the ct-major startup groups become
        # ready the moment their x^T tile lands; smalls next, proj weights
        # last.
        nc.sync.dma_start(out=w3_sb[:, 0, :, :], in_=ins["w3"][:, 0, :, :])
        nc.sync.dma_start(out=xT_sb[:, 0, 0:512], in_=ins["xT"][:, 0, 0:512])
        nc.sync.dma_start(out=xT_sb[:, 0, 512:1024], in_=ins["xT"][:, 0, 512:1024])
        nc.sync.dma_start(out=xT_sb[:, 0, 1024:T], in_=ins["xT"][:, 0, 1024:T])
        for ct in range(1, CT):
            nc.sync.dma_start(out=w3_sb[:, ct, :, :], in_=ins["w3"][:, ct, :, :])
            nc.sync.dma_start(
                out=xT_sb[:, ct, 0:1024], in_=ins["xT"][:, ct, 0:1024]
            )
            nc.sync.dma_start(
                out=xT_sb[:, ct, 1024:T], in_=ins["xT"][:, ct, 1024:T]
            )
        nc.sync.dma_start(out=vinit_sb[:, :], in_=ins["vinit"])
        nc.sync.dma_start(out=mask_sb[:, :], in_=ins["mask"])
        nc.sync.dma_start(out=bq_sb[:, :], in_=ins["bq"])
        nc.sync.dma_start(out=bk_sb[:, :], in_=ins["bk"])
        nc.sync.dma_start(out=ident_sb[:, :], in_=ins["ident"])
        nc.sync.dma_start(out=bp_sb[:, :], in_=ins["bp"])
        nc.sync.dma_start(out=wp_sb[:, :, :], in_=ins["wp"])

        # Pre-load the exp table set during the load phase (first exp
        # otherwise pays ~1.3us mid-kernel). Output is scratch.
        warm = asb.tile([1, 8], f32, tag="rec", bufs=4, name="warm")
        nc.scalar.activation(warm[0:1, :], warm_sb[0:1, 0:8], Exp, scale=1.0)

        # --- work generators: each yield is ~one PE matmul, so attention
        # blocks can pump them as fillers between their own iterations to
        # keep the (in-order) PE stream dense while ScalarE runs exp.
        from collections import deque

        work = deque()  # (name, generator, per-yield PE ns)
        done = set()

        def pump(ns):
            """Draw filler generators until ~ns of PE matmul time emitted."""
            drawn = 0.0
            while drawn < ns and work:
                name, g, cost = work[0]
                try:
                    next(g)
                    drawn += cost
                except StopIteration:
                    work.popleft()
                    done.add(name)

        def flush_to(target):
            if target in done:
                return
            while work:
                name, g, cost = work.popleft()
                for _ in g:
                    pass
                done.add(name)
                if name == target:
                    return

        def flush_all():
            while work:
                name, g, cost = work.popleft()
                for _ in g:
                    pass
                done.add(name)

        def qk_gen(dst_sb, w_of, b_sb, m, tq, nm):
            pt = ps.tile([P, 512], f32, tag="qkv", bufs=2,
                         name=f"ps_{nm}_{m}_{tq}")
            for ct in range(CT):
                nc.tensor.matmul(
                    pt[:, :],
                    w_of(ct)[:, ts(m, P)],
                    xT_sb[:, ct, ts(tq, 512)],
                    start=(ct == 0),
                    stop=(ct == CT - 1),
                )
                if ct == CT - 1:
                    nc.vector.tensor_scalar_add(
                        dst_sb[:, m, ts(tq, 512)], pt[:, :], b_sb[:, m : m + 1]
                    )
                yield

        def v_gen(t):
            pt = ps.tile([P, CPC], f32, tag="qkv", bufs=2, name=f"ps_v_{t}")
            for ct in range(CT):
                nc.tensor.matmul(
                    pt[:, :],
                    xT_sb[:, ct, ts(t, P)],
                    wv(ct),
                    start=(ct == 0),
                    stop=(ct == CT - 1),
                )
                if ct == CT - 1:
                    vslot = vext_sb[:, t, :].rearrange(
                        "p (h u) -> p h u", u=VW
                    )
                    vini = vinit_sb[:, :].rearrange("p (h u) -> p h u", u=VW)
                    nc.vector.tensor_add(
                        vslot[:, :, 0:HD],
                        pt[:, :].rearrange("p (h d) -> p h d", d=HD),
                        vini[:, :, 0:HD],
                    )
                    nc.vector.tensor_copy(
                        vslot[:, :, HD : HD + 1], vini[:, :, HD : HD + 1]
                    )
                yield

        def proj_gen(t):
            for _ in emit_yt_proj_gen(t):
                yield

        def emit_yt_proj(t, last=False):
            for _ in emit_yt_proj_gen(t, last=last):
                pass

        def emit_yt_proj_gen(t, last=False):
            """Transpose y[q, ch] tile t back to y^T via the PE (Act engine
            does the PSUM->SBUF move), then the proj matmuls + bias adds +
            per-ch output DMA. ch0 add on DVE, ch1 on the otherwise-idle
            GpSimd engine."""
            for m in range(2):
                ytp = ps.tile([P, P], bf16, tag="qkv", bufs=2,
                              name=f"ytp_{t}_{m}")
                nc.tensor.transpose(
                    ytp[:, :],
                    y_sb[:, t, m, :, :].rearrange("p a d -> p (a d)"),
                    ident_sb[:, :],
                )
                nc.vector.tensor_copy(yT_sb[:, m, ts(t, P)], ytp[:, :])
                yield
            stage = asb.tile([P, C], bf16, tag="stage", bufs=4,
                             name=f"stage_{t}")
            for ch in range(2):
                prj = ps.tile([P, 512], f32, tag="qkv", bufs=2,
                              name=f"prj_{t}_{ch}")
                for m in range(2):
                    nc.tensor.matmul(
                        prj[:, :],
                        yT_sb[:, m, ts(t, P)],
                        wp_sb[:, m, ts(ch, 512)],
                        start=(m == 0),
                        stop=(m == 1),
                    )
                nc.vector.tensor_add(
                    stage[:, ts(ch, 512)], prj[:, :], bp_sb[:, ts(ch, 512)]
                )
                nc.sync.dma_start(
                    out=out_ap[ts(t, P), ts(ch, 512)],
                    in_=stage[:, ts(ch, 512)],
                )
                yield

        def attention_block(hp, j, emit_proj=False, last=False):
            """Causal attention for 512-col q-block j of head-pair hp.

            S^T tiles [128 kpos, q] as before, but PV runs transposed:
            stationary pt q-chunk [128 k, 128 q], moving v_ext [128 k, 65]
            -> y accumulates as [128 q, 65] using all PE partitions (half
            the moving columns of the y^T orientation), and column 64 is
            the softmax denominator already transposed, so normalization
            is a per-partition reciprocal + tensor_scalar multiply -- no
            partition broadcast. Each q-chunk finalizes as soon as its
            last k-tile stops, so y/proj work pipelines inside the block
            instead of queueing after it."""
            n_tk = 4 * (j + 1)
            yps_t = [
                ps.tile([P, 2, 2, VW], f32, tag="pv", bufs=2,
                        name=f"yps_{j}_{hp}_{cc}")
                for cc in range(2)
            ]

            def yps(c, a):
                return yps_t[c // 2][:, c % 2, a, :]

            yps_started = [False, False]

            for tk in range(n_tk):
                off = max(0, P * tk - 512 * j)
                c_min = off // P
                sp = ps.tile([P, 2, 512], f32, tag="s", bufs=2,
                             name=f"s_{j}_{hp}_{tk}")
                for a in range(2):
                    lo, hi = a * 64, a * 64 + 64
                    nc.tensor.matmul(
                        sp[:, a, off:512],
                        kT_sb[lo:hi, hp, ts(tk, P)],
                        qT_sb[lo:hi, hp, 512 * j + off : 512 * (j + 1)],
                        start=True,
                        stop=True,
                    )
                pt = asb.tile([P, 2, 512], bf16, tag="pt", bufs=4,
                              name=f"pt_{j}_{hp}_{tk}")
                nc.scalar.activation(
                    pt[:, :, off:512], sp[:, :, off:512], Exp, scale=0.125
                )
                if tk >= 4 * j:  # diagonal tile: apply causal 0/1 mask on
                    # the 128-col window straddling the diagonal; columns
                    # past it are valid for every partition.
                    for a in range(2):
                        nc.vector.tensor_mul(
                            pt[:, a, off : off + P],
                            pt[:, a, off : off + P],
                            mask_sb[:, :],
                        )
                for a in range(2):
                    h = 2 * hp + a
                    for c in range(c_min, 4):
                        # start=True clears the whole PSUM *bank*, so only
                        # the first matmul into each yps tile may carry it;
                        # sibling regions start cleanly anyway because the
                        # bank clear resets per-element has_written (unset
                        # elements are overwritten, not accumulated).
                        ti = c // 2
                        st_ = tk == 0 and not yps_started[ti]
                        if st_:
                            yps_started[ti] = True
                        nc.tensor.matmul(
                            yps(c, a),
                            pt[:, a, P * c : P * (c + 1)],
                            vext_sb[:, tk, ts(h, VW)],
                            start=st_,
                            stop=(tk == 4 * j + c),
                            skip_group_check=True,
                        )
                if tk >= 4 * j:
                    c = tk - 4 * j
                    t = 4 * j + c
                    rec = asb.tile([P, 2], f32, tag="rec", bufs=4,
                                   name=f"rec_{j}_{hp}_{c}")
                    for a in range(2):
                        nc.vector.reciprocal(
                            rec[:, a : a + 1], yps(c, a)[:, HD : HD + 1]
                        )
                        nc.vector.tensor_scalar_mul(
                            y_sb[:, t, hp, a, :], yps(c, a)[:, 0:HD],
                            rec[:, a : a + 1],
                        )
                    if emit_proj and c >= 1:
                        emit_yt_proj(t - 1, last=last)
                # pump filler to cover this tile's Act-vs-PE deficit
                w = 512 - off
                act_ns = 2 * w * 0.8333 + 245
                pe_ns = (2 * w + (4 - c_min) * 2 * VW) * 0.4167
                if emit_proj and tk >= 4 * j:
                    pe_ns += 2048 * 0.4167  # embedded proj tile
                pump(act_ns - pe_ns)
            if emit_proj:
                emit_yt_proj(4 * j + 3, last=last)

        # Schedule: kick off attention (the ScalarE exp stream is the
        # attention-phase bottleneck) as soon as its inputs exist; hp0 j
        # ascending (v-availability), hp1 j descending so the final block
        # is the smallest (shortest dependency tail). proj tiles are
        # emitted per-chunk inside the hp1 blocks; hp1 q/k generators stay
        # queued as pump filler during the late attention blocks.
        # Startup: twelve passes (q/k m0 for tq0..tq2, v t0..t3)
        # interleaved ct-major, sized to fill all 8 PSUM banks, so the PE
        # has ~4096 matmul columns to run per arriving x^T tile during the
        # input-DMA wall; dummy-matmul shims between groups absorb the
        # small feed deficit so the PE clock never idles back to a cold
        # pstate. The extra passes borrow the (still idle) "s"/"pv" PSUM
        # slots.
        sq0 = ps.tile([P, 512], f32, tag="qkv", bufs=2, name="ps_q_0_0")
        sk0 = ps.tile([P, 512], f32, tag="qkv", bufs=2, name="ps_k_0_0")
        sqk1 = ps.tile([P, 2, 512], f32, tag="s", bufs=2, name="ps_qk_0_1")
        sqk2 = ps.tile([P, 2, 512], f32, tag="s", bufs=2, name="ps_qk_0_2")
        sv01 = ps.tile([P, 512], f32, tag="pv", bufs=2, name="ps_v_01")
        sv23 = ps.tile([P, 512], f32, tag="pv", bufs=2, name="ps_v_23")
        for ct in range(CT):
            st = ct == 0
            sp_ = ct == CT - 1
            # first sub-group needs x^T[ct] cols 0:1024 only
            nc.tensor.matmul(sq0[:, :], wq(ct)[:, ts(0, P)],
                             xT_sb[:, ct, ts(0, 512)], start=st, stop=sp_)
            nc.tensor.matmul(sk0[:, :], wk(ct)[:, ts(0, P)],
                             xT_sb[:, ct, ts(0, 512)], start=st, stop=sp_)
            # only the first matmul into each packed v tile carries
            # start (start=True clears the whole PSUM bank)
            nc.tensor.matmul(sv01[:, 0:CPC], xT_sb[:, ct, ts(0, P)],
                             wv(ct), start=st, stop=sp_)
            nc.tensor.matmul(sv01[:, CPC:512], xT_sb[:, ct, ts(1, P)],
                             wv(ct), start=False, stop=sp_,
                             skip_group_check=True)
            nc.tensor.matmul(sv23[:, 0:CPC], xT_sb[:, ct, ts(2, P)],
                             wv(ct), start=st, stop=sp_)
            nc.tensor.matmul(sv23[:, CPC:512], xT_sb[:, ct, ts(3, P)],
                             wv(ct), start=False, stop=sp_,
                             skip_group_check=True)
            nc.tensor.matmul(sqk1[:, 0, :], wq(ct)[:, ts(0, P)],
                             xT_sb[:, ct, ts(1, 512)], start=st, stop=sp_)
            nc.tensor.matmul(sqk1[:, 1, :], wk(ct)[:, ts(0, P)],
                             xT_sb[:, ct, ts(1, 512)], start=st, stop=sp_)
            # second sub-group needs x^T[ct] cols 1024:1536
            nc.tensor.matmul(sqk2[:, 0, :], wq(ct)[:, ts(0, P)],
                             xT_sb[:, ct, ts(2, 512)], start=st, stop=sp_)
            nc.tensor.matmul(sqk2[:, 1, :], wk(ct)[:, ts(0, P)],
                             xT_sb[:, ct, ts(2, 512)], start=st, stop=sp_)
            if ct < CT - 1:
                dummy_mm(f"shim{ct}", N_SHIM + (2 if ct < 3 else 0))
        for m_, tq_, pt_, dst_, b_ in (
            (0, 0, sq0[:, :], qT_sb, bq_sb),
            (0, 0, sk0[:, :], kT_sb, bk_sb),
            (0, 1, sqk1[:, 0, :], qT_sb, bq_sb),
            (0, 1, sqk1[:, 1, :], kT_sb, bk_sb),
            (0, 2, sqk2[:, 0, :], qT_sb, bq_sb),
            (0, 2, sqk2[:, 1, :], kT_sb, bk_sb),
        ):
            nc.vector.tensor_scalar_add(
                dst_[:, m_, ts(tq_, 512)], pt_, b_[:, m_ : m_ + 1]
            )
        vini = vinit_sb[:, :].rearrange("p (h u) -> p h u", u=VW)
        for t in range(4):
            pt_ = (sv01, sv23)[t // 2][:, ts(t % 2, CPC)]
            vslot = vext_sb[:, t, :].rearrange("p (h u) -> p h u", u=VW)
            nc.vector.tensor_add(
                vslot[:, :, 0:HD],
                pt_.rearrange("p (h d) -> p h d", d=HD),
                vini[:, :, 0:HD],
            )
            nc.vector.tensor_copy(
                vslot[:, :, HD : HD + 1], vini[:, :, HD : HD + 1]
            )

        work.append(("q_1_0", qk_gen(qT_sb, wq, bq_sb, 1, 0, "q"), 213.0))
        work.append(("k_1_0", qk_gen(kT_sb, wk, bk_sb, 1, 0, "k"), 213.0))
        for t in range(4, 8):
            work.append((f"v{t}", v_gen(t), 107.0))
        work.append(("q_1_1", qk_gen(qT_sb, wq, bq_sb, 1, 1, "q"), 213.0))
        work.append(("k_1_1", qk_gen(kT_sb, wk, bk_sb, 1, 1, "k"), 213.0))
        for t in range(8, 12):
            work.append((f"v{t}", v_gen(t), 107.0))
        work.append(("q_0_3", qk_gen(qT_sb, wq, bq_sb, 0, 3, "q"), 213.0))
        work.append(("k_0_3", qk_gen(kT_sb, wk, bk_sb, 0, 3, "k"), 213.0))
        work.append(("q_1_2", qk_gen(qT_sb, wq, bq_sb, 1, 2, "q"), 213.0))
        work.append(("k_1_2", qk_gen(kT_sb, wk, bk_sb, 1, 2, "k"), 213.0))
        for t in range(12, 16):
            work.append((f"v{t}", v_gen(t), 107.0))
        work.append(("q_1_3", qk_gen(qT_sb, wq, bq_sb, 1, 3, "q"), 213.0))
        work.append(("k_1_3", qk_gen(kT_sb, wk, bk_sb, 1, 3, "k"), 213.0))

        pump(3000)
        attention_block(0, 0)
        flush_to("k_1_0")
        attention_block(1, 0, emit_proj=True)
        flush_to("v7")
        attention_block(0, 1)
        flush_to("k_1_1")
        attention_block(1, 1, emit_proj=True)
        flush_to("v11")
        attention_block(0, 2)
        flush_to("k_1_2")
        attention_block(1, 2, emit_proj=True)
        flush_to("v15")
        attention_block(0, 3)
        flush_to("k_1_3")
        attention_block(1, 3, emit_proj=True, last=True)
        flush_all()


def _build_bass():
    import concourse.mybir as mybir
    import concourse.tile as tile
    from concourse import bacc

    f32 = mybir.dt.float32
    bf16 = mybir.dt.bfloat16
    nc = bacc.Bacc("TRN2", num_devices=NCORES)

    shapes = {
        "xT": ([P, CT, T], bf16),
        "w3": ([P, CT, 3, CPC], bf16),
        "bq": ([P, 2], f32),
        "bk": ([P, 2], f32),
        "vinit": ([P, HPC * VW], bf16),
        "mask": ([P, P], bf16),
        "ident": ([P, P], bf16),
        "wp": ([P, 2, C], bf16),
        "bp": ([P, C], bf16),
    }
    ins = {
        name: nc.dram_tensor(name, shp, dt, kind="ExternalInput").ap()
        for name, (shp, dt) in shapes.items()
    }
    out_ap = nc.dram_tensor("out", [T, C], bf16, kind="ExternalOutput").ap()

    with tile.TileContext(nc) as tc:
        _emit(tc, out_ap, ins)
    nc.compile()
    return nc


def _causal_mask_host():
    import ml_dtypes

    p = np.arange(P)[:, None]
    u = np.arange(P)[None, :]
    return (p <= u).astype(ml_dtypes.bfloat16)


def _shard(x, w_attn, b_attn, w_proj, b_proj):
    import ml_dtypes

    bf16 = ml_dtypes.bfloat16
    mask = _causal_mask_host()
    ident = np.eye(P, dtype=np.float32).astype(bf16)
    xTs = [
        np.ascontiguousarray(
            x[b].T.reshape(CT, P, T).transpose(1, 0, 2)
        ).astype(bf16)
        for b in range(B)
    ]

    def wslice(off):
        w = w_attn[:, off : off + CPC]
        return w.reshape(CT, P, CPC).transpose(1, 0, 2)

    maps = []
    for core in range(NCORES):
        b, g = divmod(core, NCORES // B)
        c0 = g * CPC
        # packed [P, CT, 3, CPC] = {wq, wk, wv} slices
        w3 = np.stack(
            [wslice(c0), wslice(C + c0), wslice(2 * C + c0)], axis=2
        ).astype(bf16)
        bv = b_attn[2 * C + c0 : 2 * C + c0 + CPC]
        vinit = np.zeros((P, HPC * VW), np.float32)
        for h in range(HPC):
            vinit[:, h * VW : h * VW + HD] = bv[h * HD : (h + 1) * HD][None, :]
            vinit[:, h * VW + HD] = 1.0
        bp = np.zeros((P, C), np.float32)
        bp[:, c0 : c0 + CPC] = b_proj[c0 : c0 + CPC][None, :]
        maps.append(
            {
                "xT": xTs[b],
                "w3": w3,
                "bq": np.ascontiguousarray(
                    b_attn[c0 : c0 + CPC].reshape(2, P).T
                ),
                "bk": np.ascontiguousarray(
                    b_attn[C + c0 : C + c0 + CPC].reshape(2, P).T
                ),
                "vinit": vinit.astype(bf16),
                "mask": mask,
                "ident": ident,
                "wp": np.ascontiguousarray(
                    w_proj[c0 : c0 + CPC, :].reshape(2, P, C).transpose(1, 0, 2)
                ).astype(bf16),
                "bp": bp.astype(bf16),
            }
        )
    return maps


TRACE = False
LAST = None


def _stub_missing_axon_hooks():
    """Some containers lack antenv.axon_hooks; stub it so trace=True
    degrades to a warning instead of crashing run_bass_kernel_spmd."""
    import sys
    import types

    try:
        import antenv.axon_hooks  # noqa: F401
    except ModuleNotFoundError:
        mod = types.ModuleType("antenv.axon_hooks")
        mod.get_axon_ntff_profile_hook = lambda: None
        sys.modules["antenv.axon_hooks"] = mod


def kernel(x, w_attn, b_attn, w_proj, b_proj):
    global LAST
    _stub_missing_axon_hooks()
    from concourse.bass_utils import run_bass_kernel_spmd

    x = np.asarray(x, np.float32)
    w_attn = np.asarray(w_attn, np.float32)
    b_attn = np.asarray(b_attn, np.float32)
    w_proj = np.asarray(w_proj, np.float32)
    b_proj = np.asarray(b_proj, np.float32)

    if "nc" not in _CACHE:
        _CACHE["nc"] = _build_bass()
    nc = _CACHE["nc"]

    in_maps = _shard(x, w_attn, b_attn, w_proj, b_proj)
    res = run_bass_kernel_spmd(
        nc, in_maps, core_ids=list(range(NCORES)), trace=TRACE
    )
    LAST = res
    out = np.zeros((B, T, C), np.float32)
    for core in range(NCORES):
        out[core // (NCORES // B)] += res.results[core]["out"].astype(np.float32)
    return out


# revision 35
# speedup vs baseline: 1.2577x; 1.0277x over previous
"""Causal self-attention (B=2, T=2048, C=1024, 16 heads) on 8 Trainium2 cores.

Sharding: data-parallel over batch (2), tensor-parallel over heads (4/core).
Core c = b*4+g handles batch b, heads [4g, 4g+4). Each core computes its
qkv slice, causal attention for its 4 heads, and a row-parallel partial of
the output projection (its 256 input channels of w_proj). The host sums the
4 partials per batch; b_proj is added on-device exactly once per column
(each core receives b_proj zero-masked to its own column quarter, host
pre-broadcast across partitions, added during the PSUM->SBUF move).

All data is bf16 (inputs rounded host-side): matmul streams at the same
1 cyc/row as fp32r but without the >=256-moving-column restriction, so
diagonal attention tiles narrow to their true width; DVE elementwise ops
on pure-SBUF bf16 run at 2x; DMA bytes halve. Partial outputs leave the
device as bf16 and are summed in f32 on the host (rel-err ~1e-3, well
under the 2e-2 gate).

Device layout (per core):
  xT   [128, 8, 2048]  x^T with channels on partitions (host pre-transposed)
  w3   [128, 8, 3, 256] packed {wq,wk,wv} column slices
  q^T/k^T computed as [128ch, 2, 2048] (2 tiles of 2 heads each)
  S^T[tk, tq] = (k^T)^T @ q^T per head; two heads packed in the 128x128 PE
  array via base-partition row groups (K=64 each). exp on ScalarE reads
  PSUM directly (scores ~ N(0,1): no max subtraction needed); causal mask
  applied only on diagonal tiles via a 0/1 mask multiply on the 128-col
  window that actually straddles the diagonal. The PV matmul uses v
  extended with a ones column -> row 64 of the PSUM accumulator is the
  softmax denominator for free.

A memset + ~26 dummy 128-col matmuls run during the initial DMA wall so
the PE clock ramp (0.65/1.2 GHz cold states in the cost model) completes
on garbage work before the first real matmul issues.
"""

import numpy as np

B, T, C = 2, 2048, 1024
NH, HD = 16, 64
NCORES = 8
HPC = 4                # heads per core
CPC = HPC * HD         # 256 channels per core
P = 128
CT = C // P            # 8 contraction tiles over C
TT = T // P            # 16 tiles of 128 over T
NTQ = T // 512         # 4 query blocks of 512
VW = HD + 1            # 65: head width in vext (v columns + ones column)
N_WARM = 26            # PE ramp-warmup matmuls (128 cols each)
N_SHIM = 3             # dummy matmuls between startup ct-groups

_CACHE = {}


def _emit(tc, out_ap, ins):
    """Emit the per-core program into TileContext tc.

    ins: dict of input APs (xT, w3, bq, bk, vinit, mask, wp, bp).
    out_ap: [T, C] partial-output DRAM AP (bf16).
    """
    import concourse.mybir as mybir
    from concourse.bass import ts

    nc = tc.nc
    f32 = mybir.dt.float32
    bf16 = mybir.dt.bfloat16
    Exp = mybir.ActivationFunctionType.Exp
    Copy = mybir.ActivationFunctionType.Copy

    with (
        tc.tile_pool(name="pers", bufs=1) as pers,
        tc.tile_pool(name="xw", bufs=1) as xw,
        tc.tile_pool(name="attn_sb", bufs=1) as asb,
        tc.tile_pool(name="ps", bufs=1, space="PSUM") as ps,
    ):
        qT_sb = pers.tile([P, 2, T], bf16, name="qT_sb")
        kT_sb = pers.tile([P, 2, T], bf16, name="kT_sb")
        yT_sb = pers.tile([P, 2, T], bf16, name="yT_sb")
        y_sb = pers.tile([P, TT, 2, 2, HD], bf16, name="y_sb")
        ident_sb = pers.tile([P, P], bf16, name="ident_sb")
        vext_sb = pers.tile([P, TT, HPC * VW], bf16, name="vext_sb")
        vinit_sb = pers.tile([P, HPC * VW], bf16, name="vinit_sb")
        mask_sb = pers.tile([P, P], bf16, name="mask_sb")
        bq_sb = pers.tile([P, 2], f32, name="bq_sb")
        bk_sb = pers.tile([P, 2], f32, name="bk_sb")
        wp_sb = pers.tile([P, 2, C], bf16, name="wp_sb")
        bp_sb = pers.tile([P, C], bf16, name="bp_sb")
        warm_sb = pers.tile([P, P], bf16, name="warm_sb")

        xT_sb = xw.tile([P, CT, T], bf16, name="xT_sb")
        w3_sb = xw.tile([P, CT, 3, CPC], bf16, name="w3_sb")

        def wq(ct):
            return w3_sb[:, ct, 0, :]

        def wk(ct):
            return w3_sb[:, ct, 1, :]

        def wv(ct):
            return w3_sb[:, ct, 2, :]

        # PE ramp warmup: memset a small SBUF tile (DVE, no deps, runs at
        # t~0), then stream dummy matmuls through the otherwise-idle PE
        # while the first input DMAs land. Outputs go to the (still idle)
        # "s"-tag PSUM slots and are never read.
        nc.vector.memset(warm_sb[:, :], 1.0)

        def dummy_mm(nm, n):
            for w in range(n):
                wt = ps.tile([P, 2, 512], f32, tag="s", bufs=2,
                             name=f"warm_{nm}_{w}")
                nc.tensor.matmul(
                    wt[:, 0, 0:P], warm_sb[:, :], warm_sb[:, :],
                    start=True, stop=True,
                )

        dummy_mm("init", N_WARM)

        # Load order: ct0 of the packed qkv weights + the first x^T block
        # split in three (the startup matmul interleave starts on them),
        # then per-ct (w3, xT halves) so the ct-major startup groups become
        # ready the moment their x^T tile lands; smalls next, proj weights
        # last.
        nc.sync.dma_start(out=w3_sb[:, 0, :, :], in_=ins["w3"][:, 0, :, :])
        nc.sync.dma_start(out=xT_sb[:, 0, 0:512], in_=ins["xT"][:, 0, 0:512])
        nc.sync.dma_start(out=xT_sb[:, 0, 512:1024], in_=ins["xT"][:, 0, 512:1024])
        nc.sync.dma_start(out=xT_sb[:, 0, 1024:T], in_=ins["xT"][:, 0, 1024:T])
        for ct in range(1, CT):
            nc.sync.dma_start(out=w3_sb[:, ct, :, :], in_=ins["w3"][:, ct, :, :])
            nc.sync.dma_start(
                out=xT_sb[:, ct, 0:1024], in_=ins["xT"][:, ct, 0:1024]
            )
            nc.sync.dma_start(
                out=xT_sb[:, ct, 1024:T], in_=ins["xT"][:, ct, 1024:T]
            )
        nc.sync.dma_start(out=vinit_sb[:, :], in_=ins["vinit"])
        nc.sync.dma_start(out=mask_sb[:, :], in_=ins["mask"])
        nc.sync.dma_start(out=bq_sb[:, :], in_=ins["bq"])
        nc.sync.dma_start(out=bk_sb[:, :], in_=ins["bk"])
        nc.sync.dma_start(out=ident_sb[:, :], in_=ins["ident"])
        nc.sync.dma_start(out=bp_sb[:, :], in_=ins["bp"])
        nc.sync.dma_start(out=wp_sb[:, :, :], in_=ins["wp"])

        # Pre-load the exp table set during the load phase (first exp
        # otherwise pays ~1.3us mid-kernel). Output is scratch.
        warm = asb.tile([1, 8], f32, tag="rec", bufs=4, name="warm")
        nc.scalar.activation(warm[0:1, :], warm_sb[0:1, 0:8], Exp, scale=1.0)

        # --- work generators: each yield is ~one PE matmul, so attention
        # blocks can pump them as fillers between their own iterations to
        # keep the (in-order) PE stream dense while ScalarE runs exp.
        from collections import deque

        work = deque()  # (name, generator, per-yield PE ns)
        done = set()

        def pump(ns):
            """Draw filler generators until ~ns of PE matmul time emitted."""
            drawn = 0.0
            while drawn < ns and work:
                name, g, cost = work[0]
                try:
                    next(g)
                    drawn += cost
                except StopIteration:
                    work.popleft()
                    done.add(name)

        def flush_to(target):
            if target in done:
                return
            while work:
                name, g, cost = work.popleft()
                for _ in g:
                    pass
                done.add(name)
                if name == target:
                    return

        def flush_all():
            while work:
                name, g, cost = work.popleft()
                for _ in g:
                    pass
                done.add(name)

        def qk_gen(dst_sb, w_of, b_sb, m, tq, nm):
            pt = ps.tile([P, 512], f32, tag="qkv", bufs=2,
                         name=f"ps_{nm}_{m}_{tq}")
            for ct in range(CT):
                nc.tensor.matmul(
                    pt[:, :],
                    w_of(ct)[:, ts(m, P)],
                    xT_sb[:, ct, ts(tq, 512)],
                    start=(ct == 0),
                    stop=(ct == CT - 1),
                )
                if ct == CT - 1:
                    nc.vector.tensor_scalar_add(
                        dst_sb[:, m, ts(tq, 512)], pt[:, :], b_sb[:, m : m + 1]
                    )
                yield

        def v_gen(t):
            pt = ps.tile([P, CPC], f32, tag="qkv", bufs=2, name=f"ps_v_{t}")
            for ct in range(CT):
                nc.tensor.matmul(
                    pt[:, :],
                    xT_sb[:, ct, ts(t, P)],
                    wv(ct),
                    start=(ct == 0),
                    stop=(ct == CT - 1),
                )
                if ct == CT - 1:
                    vslot = vext_sb[:, t, :].rearrange(
                        "p (h u) -> p h u", u=VW
                    )
                    vini = vinit_sb[:, :].rearrange("p (h u) -> p h u", u=VW)
                    nc.vector.tensor_add(
                        vslot[:, :, 0:HD],
                        pt[:, :].rearrange("p (h d) -> p h d", d=HD),
                        vini[:, :, 0:HD],
                    )
                    nc.vector.tensor_copy(
                        vslot[:, :, HD : HD + 1], vini[:, :, HD : HD + 1]
                    )
                yield

        def proj_gen(t):
            for _ in emit_yt_proj_gen(t):
                yield

        def emit_yt_proj(t, last=False):
            for _ in emit_yt_proj_gen(t, last=last):
                pass

        def emit_yt_proj_gen(t, last=False):
            """Transpose y[q, ch] tile t back to y^T via the PE, move it to
            SBUF, then the proj matmuls + bias + per-ch output DMA. In the
            steady state the moves/bias-adds run on DVE (ScalarE paces the
            exp stream); in the kernel tail (last=True) ScalarE is idle, so
            the moves go there and the bias folds into an extra rank-1
            matmul pass (ones ⊗ bp) so the PSUM->SBUF conversion is an
            Act copy instead of a DVE add."""
            for m in range(2):
                ytp = ps.tile([P, P], bf16, tag="qkv", bufs=2,
                              name=f"ytp_{t}_{m}")
                nc.tensor.transpose(
                    ytp[:, :],
                    y_sb[:, t, m, :, :].rearrange("p a d -> p (a d)"),
                    ident_sb[:, :],
                )
                nc.vector.tensor_copy(yT_sb[:, m, ts(t, P)], ytp[:, :])
                yield
            stage = asb.tile([P, C], bf16, tag="stage", bufs=4,
                             name=f"stage_{t}")
            for ch in range(2):
                prj = ps.tile([P, 512], f32, tag="qkv", bufs=2,
                              name=f"prj_{t}_{ch}")
                for m in range(2):
                    nc.tensor.matmul(
                        prj[:, :],
                        yT_sb[:, m, ts(t, P)],
                        wp_sb[:, m, ts(ch, 512)],
                        start=(m == 0),
                        stop=(m == 1) and not last,
                    )
                if last:
                    nc.tensor.matmul(
                        prj[:, :],
                        warm_sb[0:1, :],
                        bp_sb[0:1, ts(ch, 512)],
                        start=False,
                        stop=True,
                        skip_group_check=True,
                    )
                    nc.scalar.activation(
                        stage[:, ts(ch, 512)], prj[:, :], Copy, scale=1.0
                    )
                else:
                    nc.vector.tensor_add(
                        stage[:, ts(ch, 512)], prj[:, :], bp_sb[:, ts(ch, 512)]
                    )
                nc.sync.dma_start(
                    out=out_ap[ts(t, P), ts(ch, 512)],
                    in_=stage[:, ts(ch, 512)],
                )
                yield

        def attention_block(hp, j, emit_proj=False, last=False):
            """Causal attention for 512-col q-block j of head-pair hp.

            S^T tiles [128 kpos, q] as before, but PV runs transposed:
            stationary pt q-chunk [128 k, 128 q], moving v_ext [128 k, 65]
            -> y accumulates as [128 q, 65] using all PE partitions (half
            the moving columns of the y^T orientation), and column 64 is
            the softmax denominator already transposed, so normalization
            is a per-partition reciprocal + tensor_scalar multiply -- no
            partition broadcast. Each q-chunk finalizes as soon as its
            last k-tile stops, so y/proj work pipelines inside the block
            instead of queueing after it."""
            n_tk = 4 * (j + 1)
            yps_t = [
                ps.tile([P, 2, 2, VW], f32, tag="pv", bufs=2,
                        name=f"yps_{j}_{hp}_{cc}")
                for cc in range(2)
            ]

            def yps(c, a):
                return yps_t[c // 2][:, c % 2, a, :]

            yps_started = [False, False]

            for tk in range(n_tk):
                off = max(0, P * tk - 512 * j)
                c_min = off // P
                sp = ps.tile([P, 2, 512], f32, tag="s", bufs=2,
                             name=f"s_{j}_{hp}_{tk}")
                for a in range(2):
                    lo, hi = a * 64, a * 64 + 64
                    nc.tensor.matmul(
                        sp[:, a, off:512],
                        kT_sb[lo:hi, hp, ts(tk, P)],
                        qT_sb[lo:hi, hp, 512 * j + off : 512 * (j + 1)],
                        start=True,
                        stop=True,
                    )
                pt = asb.tile([P, 2, 512], bf16, tag="pt", bufs=4,
                              name=f"pt_{j}_{hp}_{tk}")
                nc.scalar.activation(
                    pt[:, :, off:512], sp[:, :, off:512], Exp, scale=0.125
                )
                if tk >= 4 * j:  # diagonal tile: apply causal 0/1 mask on
                    # the 128-col window straddling the diagonal; columns
                    # past it are valid for every partition.
                    for a in range(2):
                        nc.vector.tensor_mul(
                            pt[:, a, off : off + P],
                            pt[:, a, off : off + P],
                            mask_sb[:, :],
                        )
                for a in range(2):
                    h = 2 * hp + a
                    for c in range(c_min, 4):
                        # start=True clears the whole PSUM *bank*, so only
                        # the first matmul into each yps tile may carry it;
                        # sibling regions start cleanly anyway because the
                        # bank clear resets per-element has_written (unset
                        # elements are overwritten, not accumulated).
                        ti = c // 2
                        st_ = tk == 0 and not yps_started[ti]
                        if st_:
                            yps_started[ti] = True
                        nc.tensor.matmul(
                            yps(c, a),
                            pt[:, a, P * c : P * (c + 1)],
                            vext_sb[:, tk, ts(h, VW)],
                            start=st_,
                            stop=(tk == 4 * j + c),
                            skip_group_check=True,
                        )
                if tk >= 4 * j:
                    c = tk - 4 * j
                    t = 4 * j + c
                    rec = asb.tile([P, 2], f32, tag="rec", bufs=4,
                                   name=f"rec_{j}_{hp}_{c}")
                    for a in range(2):
                        nc.vector.reciprocal(
                            rec[:, a : a + 1], yps(c, a)[:, HD : HD + 1]
                        )
                        nc.vector.tensor_scalar_mul(
                            y_sb[:, t, hp, a, :], yps(c, a)[:, 0:HD],
                            rec[:, a : a + 1],
                        )
                    if emit_proj and c >= 1:
                        emit_yt_proj(t - 1)
                # pump filler to cover this tile's Act-vs-PE deficit
                w = 512 - off
                act_ns = 2 * w * 0.8333 + 245
                pe_ns = (2 * w + (4 - c_min) * 2 * VW) * 0.4167
                if emit_proj and tk >= 4 * j:
                    pe_ns += 2048 * 0.4167  # embedded proj tile
                pump(act_ns - pe_ns)
            if emit_proj:
                emit_yt_proj(4 * j + 3)

        # Schedule: kick off attention (the ScalarE exp stream is the
        # attention-phase bottleneck) as soon as its inputs exist; hp0 j
        # ascending (v-availability), hp1 j descending so the final block
        # is the smallest (shortest dependency tail). proj tiles are
        # emitted per-chunk inside the hp1 blocks; hp1 q/k generators stay
        # queued as pump filler during the late attention blocks.
        # Startup: twelve passes (q/k m0 for tq0..tq2, v t0..t3)
        # interleaved ct-major, sized to fill all 8 PSUM banks, so the PE
        # has ~4096 matmul columns to run per arriving x^T tile during the
        # input-DMA wall; dummy-matmul shims between groups absorb the
        # small feed deficit so the PE clock never idles back to a cold
        # pstate. The extra passes borrow the (still idle) "s"/"pv" PSUM
        # slots.
        sq0 = ps.tile([P, 512], f32, tag="qkv", bufs=2, name="ps_q_0_0")
        sk0 = ps.tile([P, 512], f32, tag="qkv", bufs=2, name="ps_k_0_0")
        sqk1 = ps.tile([P, 2, 512], f32, tag="s", bufs=2, name="ps_qk_0_1")
        sqk2 = ps.tile([P, 2, 512], f32, tag="s", bufs=2, name="ps_qk_0_2")
        sv01 = ps.tile([P, 512], f32, tag="pv", bufs=2, name="ps_v_01")
        sv23 = ps.tile([P, 512], f32, tag="pv", bufs=2, name="ps_v_23")
        for ct in range(CT):
            st = ct == 0
            sp_ = ct == CT - 1
            # first sub-group needs x^T[ct] cols 0:1024 only
            nc.tensor.matmul(sq0[:, :], wq(ct)[:, ts(0, P)],
                             xT_sb[:, ct, ts(0, 512)], start=st, stop=sp_)
            nc.tensor.matmul(sk0[:, :], wk(ct)[:, ts(0, P)],
                             xT_sb[:, ct, ts(0, 512)], start=st, stop=sp_)
            # only the first matmul into each packed v tile carries
            # start (start=True clears the whole PSUM bank)
            nc.tensor.matmul(sv01[:, 0:CPC], xT_sb[:, ct, ts(0, P)],
                             wv(ct), start=st, stop=sp_)
            nc.tensor.matmul(sv01[:, CPC:512], xT_sb[:, ct, ts(1, P)],
                             wv(ct), start=False, stop=sp_,
                             skip_group_check=True)
            nc.tensor.matmul(sv23[:, 0:CPC], xT_sb[:, ct, ts(2, P)],
                             wv(ct), start=st, stop=sp_)
            nc.tensor.matmul(sv23[:, CPC:512], xT_sb[:, ct, ts(3, P)],
                             wv(ct), start=False, stop=sp_,
                             skip_group_check=True)
            nc.tensor.matmul(sqk1[:, 0, :], wq(ct)[:, ts(0, P)],
                             xT_sb[:, ct, ts(1, 512)], start=st, stop=sp_)
            nc.tensor.matmul(sqk1[:, 1, :], wk(ct)[:, ts(0, P)],
                             xT_sb[:, ct, ts(1, 512)], start=st, stop=sp_)
            # second sub-group needs x^T[ct] cols 1024:1536
            nc.tensor.matmul(sqk2[:, 0, :], wq(ct)[:, ts(0, P)],
                             xT_sb[:, ct, ts(2, 512)], start=st, stop=sp_)
            nc.tensor.matmul(sqk2[:, 1, :], wk(ct)[:, ts(0, P)],
                             xT_sb[:, ct, ts(2, 512)], start=st, stop=sp_)
            if ct < CT - 1:
                dummy_mm(f"shim{ct}", N_SHIM + (2 if ct < 3 else 0))
        for m_, tq_, pt_, dst_, b_ in (
            (0, 0, sq0[:, :], qT_sb, bq_sb),
            (0, 0, sk0[:, :], kT_sb, bk_sb),
            (0, 1, sqk1[:, 0, :], qT_sb, bq_sb),
            (0, 1, sqk1[:, 1, :], kT_sb, bk_sb),
            (0, 2, sqk2[:, 0, :], qT_sb, bq_sb),
            (0, 2, sqk2[:, 1, :], kT_sb, bk_sb),
        ):
            nc.vector.tensor_scalar_add(
                dst_[:, m_, ts(tq_, 512)], pt_, b_[:, m_ : m_ + 1]
            )
        vini = vinit_sb[:, :].rearrange("p (h u) -> p h u", u=VW)
        for t in range(4):
            pt_ = (sv01, sv23)[t // 2][:, ts(t % 2, CPC)]
            vslot = vext_sb[:, t, :].rearrange("p (h u) -> p h u", u=VW)
            nc.vector.tensor_add(
                vslot[:, :, 0:HD],
                pt_.rearrange("p (h d) -> p h d", d=HD),
                vini[:, :, 0:HD],
            )
            nc.vector.tensor_copy(
                vslot[:, :, HD : HD + 1], vini[:, :, HD : HD + 1]
            )

        work.append(("q_1_0", qk_gen(qT_sb, wq, bq_sb, 1, 0, "q"), 213.0))
        work.append(("k_1_0", qk_gen(kT_sb, wk, bk_sb, 1, 0, "k"), 213.0))
        for t in range(4, 8):
            work.append((f"v{t}", v_gen(t), 107.0))
        work.append(("q_1_1", qk_gen(qT_sb, wq, bq_sb, 1, 1, "q"), 213.0))
        work.append(("k_1_1", qk_gen(kT_sb, wk, bk_sb, 1, 1, "k"), 213.0))
        for t in range(8, 12):
            work.append((f"v{t}", v_gen(t), 107.0))
        work.append(("q_0_3", qk_gen(qT_sb, wq, bq_sb, 0, 3, "q"), 213.0))
        work.append(("k_0_3", qk_gen(kT_sb, wk, bk_sb, 0, 3, "k"), 213.0))
        work.append(("q_1_2", qk_gen(qT_sb, wq, bq_sb, 1, 2, "q"), 213.0))
        work.append(("k_1_2", qk_gen(kT_sb, wk, bk_sb, 1, 2, "k"), 213.0))
        for t in range(12, 16):
            work.append((f"v{t}", v_gen(t), 107.0))
        work.append(("q_1_3", qk_gen(qT_sb, wq, bq_sb, 1, 3, "q"), 213.0))
        work.append(("k_1_3", qk_gen(kT_sb, wk, bk_sb, 1, 3, "k"), 213.0))

        pump(6000)
        attention_block(0, 0)
        flush_to("k_1_0")
        attention_block(1, 0, emit_proj=True)
        flush_to("v7")
        attention_block(0, 1)
        flush_to("k_1_1")
        attention_block(1, 1, emit_proj=True)
        flush_to("v11")
        attention_block(0, 2)
        flush_to("k_1_2")
        attention_block(1, 2, emit_proj=True)
        flush_to("v15")
        attention_block(0, 3)
        flush_to("k_1_3")
        attention_block(1, 3, emit_proj=True, last=True)
        flush_all()


def _build_bass():
    import concourse.mybir as mybir
    import concourse.tile as tile
    from concourse import bacc

    f32 = mybir.dt.float32
    bf16 = mybir.dt.bfloat16
    nc = bacc.Bacc("TRN2", num_devices=NCORES)

    shapes = {
        "xT": ([P, CT, T], bf16),
        "w3": ([P, CT, 3, CPC], bf16),
        "bq": ([P, 2], f32),
        "bk": ([P, 2], f32),
        "vinit": ([P, HPC * VW], bf16),
        "mask": ([P, P], bf16),
        "ident": ([P, P], bf16),
        "wp": ([P, 2, C], bf16),
        "bp": ([P, C], bf16),
    }
    ins = {
        name: nc.dram_tensor(name, shp, dt, kind="ExternalInput").ap()
        for name, (shp, dt) in shapes.items()
    }
    out_ap = nc.dram_tensor("out", [T, C], bf16, kind="ExternalOutput").ap()

    with tile.TileContext(nc) as tc:
        _emit(tc, out_ap, ins)
    nc.compile()
    return nc


def _causal_mask_host():
    import ml_dtypes

    p = np.arange(P)[:, None]
    u = np.arange(P)[None, :]
    return (p <= u).astype(ml_dtypes.bfloat16)


def _shard(x, w_attn, b_attn, w_proj, b_proj):
    import ml_dtypes

    bf16 = ml_dtypes.bfloat16
    mask = _causal_mask_host()
    ident = np.eye(P, dtype=np.float32).astype(bf16)
    xTs = [
        np.ascontiguousarray(
            x[b].T.reshape(CT, P, T).transpose(1, 0, 2)
        ).astype(bf16)
        for b in range(B)
    ]

    def wslice(off):
        w = w_attn[:, off : off + CPC]
        return w.reshape(CT, P, CPC).transpose(1, 0, 2)

    maps = []
    for core in range(NCORES):
        b, g = divmod(core, NCORES // B)
        c0 = g * CPC
        # packed [P, CT, 3, CPC] = {wq, wk, wv} slices
        w3 = np.stack(
            [wslice(c0), wslice(C + c0), wslice(2 * C + c0)], axis=2
        ).astype(bf16)
        bv = b_attn[2 * C + c0 : 2 * C + c0 + CPC]
        vinit = np.zeros((P, HPC * VW), np.float32)
        for h in range(HPC):
            vinit[:, h * VW : h * VW + HD] = bv[h * HD : (h + 1) * HD][None, :]
            vinit[:, h * VW + HD] = 1.0
        bp = np.zeros((P, C), np.float32)
        bp[:, c0 : c0 + CPC] = b_proj[c0 : c0 + CPC][None, :]
        maps.append(
            {
                "xT": xTs[b],
                "w3": w3,
                "bq": np.ascontiguousarray(
                    b_attn[c0 : c0 + CPC].reshape(2, P).T
                ),
                "bk": np.ascontiguousarray(
                    b_attn[C + c0 : C + c0 + CPC].reshape(2, P).T
                ),
                "vinit": vinit.astype(bf16),
                "mask": mask,
                "ident": ident,
                "wp": np.ascontiguousarray(
                    w_proj[c0 : c0 + CPC, :].reshape(2, P, C).transpose(1, 0, 2)
                ).astype(bf16),
                "bp": bp.astype(bf16),
            }
        )
    return maps


TRACE = False
LAST = None


def _stub_missing_axon_hooks():
    """Some containers lack antenv.axon_hooks; stub it so trace=True
    degrades to a warning instead of crashing run_bass_kernel_spmd."""
    import sys
    import types

    try:
        import antenv.axon_hooks  # noqa: F401
    except ModuleNotFoundError:
        mod = types.ModuleType("antenv.axon_hooks")
        mod.get_axon_ntff_profile_hook = lambda: None
        sys.modules["antenv.axon_hooks"] = mod


def kernel(x, w_attn, b_attn, w_proj, b_proj):
    global LAST
    _stub_missing_axon_hooks()
    from concourse.bass_utils import run_bass_kernel_spmd

    x = np.asarray(x, np.float32)
    w_attn = np.asarray(w_attn, np.float32)
    b_attn = np.asarray(b_attn, np.float32)
    w_proj = np.asarray(w_proj, np.float32)
    b_proj = np.asarray(b_proj, np.float32)

    if "nc" not in _CACHE:
        _CACHE["nc"] = _build_bass()
    nc = _CACHE["nc"]

    in_maps = _shard(x, w_attn, b_attn, w_proj, b_proj)
    res = run_bass_kernel_spmd(
        nc, in_maps, core_ids=list(range(NCORES)), trace=TRACE
    )
    LAST = res
    out = np.zeros((B, T, C), np.float32)
    for core in range(NCORES):
        out[core // (NCORES // B)] += res.results[core]["out"].astype(np.float32)
    return out


# revision 47
# speedup vs baseline: 1.3384x; 1.0642x over previous
"""Causal self-attention (B=2, T=2048, C=1024, 16 heads) on 8 Trainium2 cores.

Sharding: data-parallel over batch (2), tensor-parallel over heads (4/core).
Core c = b*4+g handles batch b, heads [4g, 4g+4). Each core computes its
qkv slice, causal attention for its 4 heads, and a row-parallel partial of
the output projection (its 256 input channels of w_proj). The host sums the
4 partials per batch; b_proj is added on-device exactly once per column
(each core receives b_proj zero-masked to its own column quarter, host
pre-broadcast across partitions, added during the PSUM->SBUF move).

All data is bf16 (inputs rounded host-side): matmul streams at the same
1 cyc/row as fp32r but without the >=256-moving-column restriction, so
diagonal attention tiles narrow to their true width; DVE elementwise ops
on pure-SBUF bf16 run at 2x; DMA bytes halve. Partial outputs leave the
device as bf16 and are summed in f32 on the host (rel-err ~1e-3, well
under the 2e-2 gate).

Device layout (per core):
  xT   [128, 8, 2048]  x^T with channels on partitions (host pre-transposed)
  w3   [128, 8, 3, 256] packed {wq,wk,wv} column slices
  q^T/k^T computed as [128ch, 2, 2048] (2 tiles of 2 heads each)
  S^T[tk, tq] = (k^T)^T @ q^T per head; two heads packed in the 128x128 PE
  array via base-partition row groups (K=64 each). exp on ScalarE reads
  PSUM directly (scores ~ N(0,1): no max subtraction needed); causal mask
  applied only on diagonal tiles via a 0/1 mask multiply on the 128-col
  window that actually straddles the diagonal. The PV matmul uses v
  extended with a ones column -> row 64 of the PSUM accumulator is the
  softmax denominator for free.

A memset + ~26 dummy 128-col matmuls run during the initial DMA wall so
the PE clock ramp (0.65/1.2 GHz cold states in the cost model) completes
on garbage work before the first real matmul issues.
"""

import numpy as np

B, T, C = 2, 2048, 1024
NH, HD = 16, 64
NCORES = 8
HPC = 4                # heads per core
CPC = HPC * HD         # 256 channels per core
P = 128
CT = C // P            # 8 contraction tiles over C
TT = T // P            # 16 tiles of 128 over T
NTQ = T // 512         # 4 query blocks of 512
VW = HD + 1            # 65: head width in vext (v columns + ones column)
N_WARM = 26            # PE ramp-warmup matmuls (128 cols each)
N_SHIM = 3             # dummy matmuls between startup ct-groups

_CACHE = {}


def _emit(tc, out_ap, ins):
    """Emit the per-core program into TileContext tc.

    ins: dict of input APs (xT, w3, bq, bk, vinit, mask, wp, bp).
    out_ap: [T, C] partial-output DRAM AP (bf16).
    """
    import concourse.mybir as mybir
    from concourse.bass import ts

    nc = tc.nc
    f32 = mybir.dt.float32
    bf16 = mybir.dt.bfloat16
    Exp = mybir.ActivationFunctionType.Exp
    Copy = mybir.ActivationFunctionType.Copy

    with (
        tc.tile_pool(name="pers", bufs=1) as pers,
        tc.tile_pool(name="xw", bufs=1) as xw,
        tc.tile_pool(name="attn_sb", bufs=1) as asb,
        tc.tile_pool(name="ps", bufs=1, space="PSUM") as ps,
    ):
        qT_sb = pers.tile([P, 2, T], bf16, name="qT_sb")
        kT_sb = pers.tile([P, 2, T], bf16, name="kT_sb")
        yT_sb = pers.tile([P, 2, T], bf16, name="yT_sb")
        y_sb = pers.tile([P, TT, 2, 2, HD], bf16, name="y_sb")
        ident_sb = pers.tile([P, P], bf16, name="ident_sb")
        vext_sb = pers.tile([P, TT, HPC * VW], bf16, name="vext_sb")
        vinit_sb = pers.tile([P, HPC * VW], bf16, name="vinit_sb")
        mask_sb = pers.tile([P, P], bf16, name="mask_sb")
        bq_sb = pers.tile([P, 2], f32, name="bq_sb")
        bk_sb = pers.tile([P, 2], f32, name="bk_sb")
        wp_sb = pers.tile([P, 2, C], bf16, name="wp_sb")
        bp_sb = pers.tile([P, C], bf16, name="bp_sb")
        warm_sb = pers.tile([P, P], bf16, name="warm_sb")

        xT_sb = xw.tile([P, CT, T], bf16, name="xT_sb")
        w3_sb = xw.tile([P, CT, 3, CPC], bf16, name="w3_sb")

        def wq(ct):
            return w3_sb[:, ct, 0, :]

        def wk(ct):
            return w3_sb[:, ct, 1, :]

        def wv(ct):
            return w3_sb[:, ct, 2, :]

        # PE ramp warmup: memset a small SBUF tile (DVE, no deps, runs at
        # t~0), then stream dummy matmuls through the otherwise-idle PE
        # while the first input DMAs land. Outputs go to the (still idle)
        # "s"-tag PSUM slots and are never read.
        nc.vector.memset(warm_sb[:, :], 1.0)

        def dummy_mm(nm, n):
            for w in range(n):
                wt = ps.tile([P, 2, 512], f32, tag="s", bufs=2,
                             name=f"warm_{nm}_{w}")
                nc.tensor.matmul(
                    wt[:, 0, 0:P], warm_sb[:, :], warm_sb[:, :],
                    start=True, stop=True,
                )

        dummy_mm("init", N_WARM)

        # Load order: ct0 of the packed qkv weights + the first x^T block
        # split in three (the startup matmul interleave starts on them),
        # then per-ct (w3, xT halves) so the ct-major startup groups become
        # ready the moment their x^T tile lands; smalls next, proj weights
        # last.
        nc.sync.dma_start(out=w3_sb[:, 0, :, :], in_=ins["w3"][:, 0, :, :])
        nc.sync.dma_start(out=xT_sb[:, 0, 0:512], in_=ins["xT"][:, 0, 0:512])
        nc.sync.dma_start(out=xT_sb[:, 0, 512:1024], in_=ins["xT"][:, 0, 512:1024])
        nc.sync.dma_start(out=xT_sb[:, 0, 1024:T], in_=ins["xT"][:, 0, 1024:T])
        for ct in range(1, CT):
            nc.sync.dma_start(out=w3_sb[:, ct, :, :], in_=ins["w3"][:, ct, :, :])
            nc.sync.dma_start(
                out=xT_sb[:, ct, 0:1024], in_=ins["xT"][:, ct, 0:1024]
            )
            nc.sync.dma_start(
                out=xT_sb[:, ct, 1024:T], in_=ins["xT"][:, ct, 1024:T]
            )
        nc.sync.dma_start(out=vinit_sb[:, :], in_=ins["vinit"])
        nc.sync.dma_start(out=mask_sb[:, :], in_=ins["mask"])
        nc.sync.dma_start(out=bq_sb[:, :], in_=ins["bq"])
        nc.sync.dma_start(out=bk_sb[:, :], in_=ins["bk"])
        nc.sync.dma_start(out=ident_sb[:, :], in_=ins["ident"])
        nc.sync.dma_start(out=bp_sb[:, :], in_=ins["bp"])
        nc.sync.dma_start(out=wp_sb[:, :, :], in_=ins["wp"])

        # Pre-load the exp table set during the load phase (first exp
        # otherwise pays ~1.3us mid-kernel). Output is scratch.
        warm = asb.tile([1, 8], f32, tag="rec", bufs=4, name="warm")
        nc.scalar.activation(warm[0:1, :], warm_sb[0:1, 0:8], Exp, scale=1.0)

        # --- work generators: each yield is ~one PE matmul, so attention
        # blocks can pump them as fillers between their own iterations to
        # keep the (in-order) PE stream dense while ScalarE runs exp.
        from collections import deque

        work = deque()  # (name, generator, per-yield PE ns)
        done = set()

        def pump(ns):
            """Draw filler generators until ~ns of PE matmul time emitted."""
            drawn = 0.0
            while drawn < ns and work:
                name, g, cost = work[0]
                try:
                    next(g)
                    drawn += cost
                except StopIteration:
                    work.popleft()
                    done.add(name)

        def flush_to(target):
            if target in done:
                return
            while work:
                name, g, cost = work.popleft()
                for _ in g:
                    pass
                done.add(name)
                if name == target:
                    return

        def flush_all():
            while work:
                name, g, cost = work.popleft()
                for _ in g:
                    pass
                done.add(name)

        def qk_gen(dst_sb, w_of, b_sb, m, tq, nm):
            pt = ps.tile([P, 512], f32, tag="qkv", bufs=2,
                         name=f"ps_{nm}_{m}_{tq}")
            for ct in range(CT):
                nc.tensor.matmul(
                    pt[:, :],
                    w_of(ct)[:, ts(m, P)],
                    xT_sb[:, ct, ts(tq, 512)],
                    start=(ct == 0),
                    stop=(ct == CT - 1),
                )
                if ct == CT - 1:
                    nc.vector.tensor_scalar_add(
                        dst_sb[:, m, ts(tq, 512)], pt[:, :], b_sb[:, m : m + 1]
                    )
                yield

        def v_gen(t):
            pt = ps.tile([P, CPC], f32, tag="qkv", bufs=2, name=f"ps_v_{t}")
            for ct in range(CT):
                nc.tensor.matmul(
                    pt[:, :],
                    xT_sb[:, ct, ts(t, P)],
                    wv(ct),
                    start=(ct == 0),
                    stop=(ct == CT - 1),
                )
                if ct == CT - 1:
                    vslot = vext_sb[:, t, :].rearrange(
                        "p (h u) -> p h u", u=VW
                    )
                    vini = vinit_sb[:, :].rearrange("p (h u) -> p h u", u=VW)
                    nc.vector.tensor_add(
                        vslot[:, :, 0:HD],
                        pt[:, :].rearrange("p (h d) -> p h d", d=HD),
                        vini[:, :, 0:HD],
                    )
                    nc.vector.tensor_copy(
                        vslot[:, :, HD : HD + 1], vini[:, :, HD : HD + 1]
                    )
                yield

        def proj_gen(t):
            for _ in emit_yt_proj_gen(t):
                yield

        def emit_yt_proj(t, last=False):
            for _ in emit_yt_proj_gen(t, last=last):
                pass

        def emit_yt_proj_gen(t, last=False):
            """Transpose y[q, ch] tile t back to y^T via the PE, move it to
            SBUF, then the proj matmuls + bias + per-ch output DMA. In the
            steady state the moves/bias-adds run on DVE (ScalarE paces the
            exp stream); in the kernel tail (last=True) ScalarE is idle, so
            the moves go there and the bias folds into an extra rank-1
            matmul pass (ones ⊗ bp) so the PSUM->SBUF conversion is an
            Act copy instead of a DVE add."""
            for m in range(2):
                ytp = ps.tile([P, P], bf16, tag="qkv", bufs=2,
                              name=f"ytp_{t}_{m}")
                nc.tensor.transpose(
                    ytp[:, :],
                    y_sb[:, t, m, :, :].rearrange("p a d -> p (a d)"),
                    ident_sb[:, :],
                )
                nc.vector.tensor_copy(yT_sb[:, m, ts(t, P)], ytp[:, :])
                yield
            stage = asb.tile([P, C], bf16, tag="stage", bufs=4,
                             name=f"stage_{t}")
            for ch in range(2):
                prj = ps.tile([P, 512], f32, tag="qkv", bufs=2,
                              name=f"prj_{t}_{ch}")
                for m in range(2):
                    nc.tensor.matmul(
                        prj[:, :],
                        yT_sb[:, m, ts(t, P)],
                        wp_sb[:, m, ts(ch, 512)],
                        start=(m == 0),
                        stop=(m == 1) and not last,
                    )
                if last:
                    nc.tensor.matmul(
                        prj[:, :],
                        warm_sb[0:1, :],
                        bp_sb[0:1, ts(ch, 512)],
                        start=False,
                        stop=True,
                        skip_group_check=True,
                    )
                    nc.scalar.activation(
                        stage[:, ts(ch, 512)], prj[:, :], Copy, scale=1.0
                    )
                else:
                    nc.vector.tensor_add(
                        stage[:, ts(ch, 512)], prj[:, :], bp_sb[:, ts(ch, 512)]
                    )
                nc.sync.dma_start(
                    out=out_ap[ts(t, P), ts(ch, 512)],
                    in_=stage[:, ts(ch, 512)],
                )
                yield

        def attention_block(hp, j, emit_proj=False, last=False):
            """Causal attention for 512-col q-block j of head-pair hp.

            S^T tiles [128 kpos, q] as before, but PV runs transposed:
            stationary pt q-chunk [128 k, 128 q], moving v_ext [128 k, 65]
            -> y accumulates as [128 q, 65] using all PE partitions (half
            the moving columns of the y^T orientation), and column 64 is
            the softmax denominator already transposed, so normalization
            is a per-partition reciprocal + tensor_scalar multiply -- no
            partition broadcast. Each q-chunk finalizes as soon as its
            last k-tile stops, so y/proj work pipelines inside the block
            instead of queueing after it."""
            n_tk = 4 * (j + 1)
            yps_t = [
                ps.tile([P, 2, 2, VW], f32, tag="pv", bufs=2,
                        name=f"yps_{j}_{hp}_{cc}")
                for cc in range(2)
            ]

            def yps(c, a):
                return yps_t[c // 2][:, c % 2, a, :]

            yps_started = [False, False]

            for tk in range(n_tk):
                off = max(0, P * tk - 512 * j)
                c_min = off // P
                sp = ps.tile([P, 2, 512], f32, tag="s", bufs=2,
                             name=f"s_{j}_{hp}_{tk}")
                for a in range(2):
                    lo, hi = a * 64, a * 64 + 64
                    nc.tensor.matmul(
                        sp[:, a, off:512],
                        kT_sb[lo:hi, hp, ts(tk, P)],
                        qT_sb[lo:hi, hp, 512 * j + off : 512 * (j + 1)],
                        start=True,
                        stop=True,
                    )
                pt = asb.tile([P, 2, 512], bf16, tag="pt", bufs=4,
                              name=f"pt_{j}_{hp}_{tk}")
                nc.scalar.activation(
                    pt[:, :, off:512], sp[:, :, off:512], Exp, scale=0.125
                )
                if tk >= 4 * j:  # diagonal tile: apply causal 0/1 mask on
                    # the 128-col window straddling the diagonal; columns
                    # past it are valid for every partition.
                    for a in range(2):
                        nc.vector.tensor_mul(
                            pt[:, a, off : off + P],
                            pt[:, a, off : off + P],
                            mask_sb[:, :],
                        )
                for a in range(2):
                    h = 2 * hp + a
                    for c in range(c_min, 4):
                        # start=True clears the whole PSUM *bank*, so only
                        # the first matmul into each yps tile may carry it;
                        # sibling regions start cleanly anyway because the
                        # bank clear resets per-element has_written (unset
                        # elements are overwritten, not accumulated).
                        ti = c // 2
                        st_ = tk == 0 and not yps_started[ti]
                        if st_:
                            yps_started[ti] = True
                        nc.tensor.matmul(
                            yps(c, a),
                            pt[:, a, P * c : P * (c + 1)],
                            vext_sb[:, tk, ts(h, VW)],
                            start=st_,
                            stop=(tk == 4 * j + c),
                            skip_group_check=True,
                        )
                if tk >= 4 * j:
                    c = tk - 4 * j
                    t = 4 * j + c
                    rec = asb.tile([P, 2], f32, tag="rec", bufs=4,
                                   name=f"rec_{j}_{hp}_{c}")
                    for a in range(2):
                        nc.vector.reciprocal(
                            rec[:, a : a + 1], yps(c, a)[:, HD : HD + 1]
                        )
                        nc.vector.tensor_scalar_mul(
                            y_sb[:, t, hp, a, :], yps(c, a)[:, 0:HD],
                            rec[:, a : a + 1],
                        )
                    if emit_proj and c >= 1:
                        emit_yt_proj(t - 1)
                # pump filler to cover this tile's Act-vs-PE deficit
                w = 512 - off
                act_ns = 2 * w * 0.8333 + 245
                pe_ns = (2 * w + (4 - c_min) * 2 * VW) * 0.4167
                if emit_proj and tk >= 4 * j:
                    pe_ns += 2048 * 0.4167  # embedded proj tile
                pump(act_ns - pe_ns)
            if emit_proj:
                emit_yt_proj(4 * j + 3)

        # Schedule: kick off attention (the ScalarE exp stream is the
        # attention-phase bottleneck) as soon as its inputs exist; hp0 j
        # ascending (v-availability), hp1 j descending so the final block
        # is the smallest (shortest dependency tail). proj tiles are
        # emitted per-chunk inside the hp1 blocks; hp1 q/k generators stay
        # queued as pump filler during the late attention blocks.
        # Startup: twelve passes (q/k m0 for tq0..tq2, v t0..t3)
        # interleaved ct-major, sized to fill all 8 PSUM banks, so the PE
        # has ~4096 matmul columns to run per arriving x^T tile during the
        # input-DMA wall; dummy-matmul shims between groups absorb the
        # small feed deficit so the PE clock never idles back to a cold
        # pstate. The extra passes borrow the (still idle) "s"/"pv" PSUM
        # slots.
        sq0 = ps.tile([P, 512], f32, tag="qkv", bufs=2, name="ps_q_0_0")
        sk0 = ps.tile([P, 512], f32, tag="qkv", bufs=2, name="ps_k_0_0")
        sqk1 = ps.tile([P, 2, 512], f32, tag="s", bufs=2, name="ps_qk_0_1")
        sqk2 = ps.tile([P, 2, 512], f32, tag="s", bufs=2, name="ps_qk_0_2")
        sv01 = ps.tile([P, 512], f32, tag="pv", bufs=2, name="ps_v_01")
        sv23 = ps.tile([P, 512], f32, tag="pv", bufs=2, name="ps_v_23")
        for ct in range(CT):
            st = ct == 0
            sp_ = ct == CT - 1
            # first sub-group needs x^T[ct] cols 0:1024 only
            nc.tensor.matmul(sq0[:, :], wq(ct)[:, ts(0, P)],
                             xT_sb[:, ct, ts(0, 512)], start=st, stop=sp_)
            nc.tensor.matmul(sk0[:, :], wk(ct)[:, ts(0, P)],
                             xT_sb[:, ct, ts(0, 512)], start=st, stop=sp_)
            # only the first matmul into each packed v tile carries
            # start (start=True clears the whole PSUM bank)
            nc.tensor.matmul(sv01[:, 0:CPC], xT_sb[:, ct, ts(0, P)],
                             wv(ct), start=st, stop=sp_)
            nc.tensor.matmul(sv01[:, CPC:512], xT_sb[:, ct, ts(1, P)],
                             wv(ct), start=False, stop=sp_,
                             skip_group_check=True)
            nc.tensor.matmul(sv23[:, 0:CPC], xT_sb[:, ct, ts(2, P)],
                             wv(ct), start=st, stop=sp_)
            nc.tensor.matmul(sv23[:, CPC:512], xT_sb[:, ct, ts(3, P)],
                             wv(ct), start=False, stop=sp_,
                             skip_group_check=True)
            nc.tensor.matmul(sqk1[:, 0, :], wq(ct)[:, ts(0, P)],
                             xT_sb[:, ct, ts(1, 512)], start=st, stop=sp_)
            nc.tensor.matmul(sqk1[:, 1, :], wk(ct)[:, ts(0, P)],
                             xT_sb[:, ct, ts(1, 512)], start=st, stop=sp_)
            # second sub-group needs x^T[ct] cols 1024:1536
            nc.tensor.matmul(sqk2[:, 0, :], wq(ct)[:, ts(0, P)],
                             xT_sb[:, ct, ts(2, 512)], start=st, stop=sp_)
            nc.tensor.matmul(sqk2[:, 1, :], wk(ct)[:, ts(0, P)],
                             xT_sb[:, ct, ts(2, 512)], start=st, stop=sp_)
            if ct < CT - 1:
                dummy_mm(f"shim{ct}", N_SHIM + (2 if ct < 3 else 0))
        for m_, tq_, pt_, dst_, b_ in (
            (0, 0, sq0[:, :], qT_sb, bq_sb),
            (0, 0, sk0[:, :], kT_sb, bk_sb),
            (0, 1, sqk1[:, 0, :], qT_sb, bq_sb),
            (0, 1, sqk1[:, 1, :], kT_sb, bk_sb),
            (0, 2, sqk2[:, 0, :], qT_sb, bq_sb),
            (0, 2, sqk2[:, 1, :], kT_sb, bk_sb),
        ):
            nc.vector.tensor_scalar_add(
                dst_[:, m_, ts(tq_, 512)], pt_, b_[:, m_ : m_ + 1]
            )
        vini = vinit_sb[:, :].rearrange("p (h u) -> p h u", u=VW)
        for t in range(4):
            pt_ = (sv01, sv23)[t // 2][:, ts(t % 2, CPC)]
            vslot = vext_sb[:, t, :].rearrange("p (h u) -> p h u", u=VW)
            nc.vector.tensor_add(
                vslot[:, :, 0:HD],
                pt_.rearrange("p (h d) -> p h d", d=HD),
                vini[:, :, 0:HD],
            )
            nc.vector.tensor_copy(
                vslot[:, :, HD : HD + 1], vini[:, :, HD : HD + 1]
            )

        work.append(("q_1_0", qk_gen(qT_sb, wq, bq_sb, 1, 0, "q"), 213.0))
        work.append(("k_1_0", qk_gen(kT_sb, wk, bk_sb, 1, 0, "k"), 213.0))
        for t in range(4, 8):
            work.append((f"v{t}", v_gen(t), 107.0))
        work.append(("q_1_1", qk_gen(qT_sb, wq, bq_sb, 1, 1, "q"), 213.0))
        work.append(("k_1_1", qk_gen(kT_sb, wk, bk_sb, 1, 1, "k"), 213.0))
        for t in range(8, 12):
            work.append((f"v{t}", v_gen(t), 107.0))
        work.append(("q_0_3", qk_gen(qT_sb, wq, bq_sb, 0, 3, "q"), 213.0))
        work.append(("k_0_3", qk_gen(kT_sb, wk, bk_sb, 0, 3, "k"), 213.0))
        work.append(("q_1_2", qk_gen(qT_sb, wq, bq_sb, 1, 2, "q"), 213.0))
        work.append(("k_1_2", qk_gen(kT_sb, wk, bk_sb, 1, 2, "k"), 213.0))
        for t in range(12, 16):
            work.append((f"v{t}", v_gen(t), 107.0))
        work.append(("q_1_3", qk_gen(qT_sb, wq, bq_sb, 1, 3, "q"), 213.0))
        work.append(("k_1_3", qk_gen(kT_sb, wk, bk_sb, 1, 3, "k"), 213.0))

        pump(3500)
        attention_block(0, 0)
        flush_to("k_1_0")
        attention_block(1, 0)
        for t in range(0, 4):
            work.append((f"p{t}", proj_gen(t), 430.0))
        flush_to("v7")
        attention_block(0, 1)
        flush_to("k_1_1")
        attention_block(1, 1)
        for t in range(4, 8):
            work.append((f"p{t}", proj_gen(t), 430.0))
        flush_to("v11")
        attention_block(0, 2)
        flush_to("k_1_2")
        attention_block(1, 2)
        for t in range(8, 12):
            work.append((f"p{t}", proj_gen(t), 430.0))
        flush_to("v15")
        attention_block(0, 3)
        flush_to("k_1_3")
        attention_block(1, 3, emit_proj=True, last=True)
        flush_all()


def _build_bass():
    import concourse.mybir as mybir
    import concourse.tile as tile
    from concourse import bacc

    f32 = mybir.dt.float32
    bf16 = mybir.dt.bfloat16
    nc = bacc.Bacc("TRN2", num_devices=NCORES)

    shapes = {
        "xT": ([P, CT, T], bf16),
        "w3": ([P, CT, 3, CPC], bf16),
        "bq": ([P, 2], f32),
        "bk": ([P, 2], f32),
        "vinit": ([P, HPC * VW], bf16),
        "mask": ([P, P], bf16),
        "ident": ([P, P], bf16),
        "wp": ([P, 2, C], bf16),
        "bp": ([P, C], bf16),
    }
    ins = {
        name: nc.dram_tensor(name, shp, dt, kind="ExternalInput").ap()
        for name, (shp, dt) in shapes.items()
    }
    out_ap = nc.dram_tensor("out", [T, C], bf16, kind="ExternalOutput").ap()

    with tile.TileContext(nc) as tc:
        _emit(tc, out_ap, ins)
    nc.compile()
    return nc


def _causal_mask_host():
    import ml_dtypes

    p = np.arange(P)[:, None]
    u = np.arange(P)[None, :]
    return (p <= u).astype(ml_dtypes.bfloat16)


def _shard(x, w_attn, b_attn, w_proj, b_proj):
    import ml_dtypes

    bf16 = ml_dtypes.bfloat16
    mask = _causal_mask_host()
    ident = np.eye(P, dtype=np.float32).astype(bf16)
    xTs = [
        np.ascontiguousarray(
            x[b].T.reshape(CT, P, T).transpose(1, 0, 2)
        ).astype(bf16)
        for b in range(B)
    ]

    def wslice(off):
        w = w_attn[:, off : off + CPC]
        return w.reshape(CT, P, CPC).transpose(1, 0, 2)

    maps = []
    for core in range(NCORES):
        b, g = divmod(core, NCORES // B)
        c0 = g * CPC
        # packed [P, CT, 3, CPC] = {wq, wk, wv} slices
        w3 = np.stack(
            [wslice(c0), wslice(C + c0), wslice(2 * C + c0)], axis=2
        ).astype(bf16)
        bv = b_attn[2 * C + c0 : 2 * C + c0 + CPC]
        vinit = np.zeros((P, HPC * VW), np.float32)
        for h in range(HPC):
            vinit[:, h * VW : h * VW + HD] = bv[h * HD : (h + 1) * HD][None, :]
            vinit[:, h * VW + HD] = 1.0
        bp = np.zeros((P, C), np.float32)
        bp[:, c0 : c0 + CPC] = b_proj[c0 : c0 + CPC][None, :]
        maps.append(
            {
                "xT": xTs[b],
                "w3": w3,
                "bq": np.ascontiguousarray(
                    b_attn[c0 : c0 + CPC].reshape(2, P).T
                ),
                "bk": np.ascontiguousarray(
                    b_attn[C + c0 : C + c0 + CPC].reshape(2, P).T
                ),
                "vinit": vinit.astype(bf16),
                "mask": mask,
                "ident": ident,
                "wp": np.ascontiguousarray(
                    w_proj[c0 : c0 + CPC, :].reshape(2, P, C).transpose(1, 0, 2)
                ).astype(bf16),
                "bp": bp.astype(bf16),
            }
        )
    return maps


TRACE = False
LAST = None


def _stub_missing_axon_hooks():
    """Some containers lack antenv.axon_hooks; stub it so trace=True
    degrades to a warning instead of crashing run_bass_kernel_spmd."""
    import sys
    import types

    try:
        import antenv.axon_hooks  # noqa: F401
    except ModuleNotFoundError:
        mod = types.ModuleType("antenv.axon_hooks")
        mod.get_axon_ntff_profile_hook = lambda: None
        sys.modules["antenv.axon_hooks"] = mod


def kernel(x, w_attn, b_attn, w_proj, b_proj):
    global LAST
    _stub_missing_axon_hooks()
    from concourse.bass_utils import run_bass_kernel_spmd

    x = np.asarray(x, np.float32)
    w_attn = np.asarray(w_attn, np.float32)
    b_attn = np.asarray(b_attn, np.float32)
    w_proj = np.asarray(w_proj, np.float32)
    b_proj = np.asarray(b_proj, np.float32)

    if "nc" not in _CACHE:
        _CACHE["nc"] = _build_bass()
    nc = _CACHE["nc"]

    in_maps = _shard(x, w_attn, b_attn, w_proj, b_proj)
    res = run_bass_kernel_spmd(
        nc, in_maps, core_ids=list(range(NCORES)), trace=TRACE
    )
    LAST = res
    out = np.zeros((B, T, C), np.float32)
    for core in range(NCORES):
        out[core // (NCORES // B)] += res.results[core]["out"].astype(np.float32)
    return out
